# revision 45
# baseline (speedup 1.0000x reference)
"""3-layer GCN on 8 Trainium2 NeuronCores — aggregate-then-transform, v5.

Math (verified vs reference in f64):
  u_l = Agg(h_l),  Agg(v)_i = dinv_i*(sum_{j->i} dinv_j v_j) + dinv_i^2 v_i
  z1 = relu(u0@W1 + b1);  h_l = a_l*z_l + bv_l  (BatchNorm folded)
  z2 = relu(u1@W2 + b2);  out = relu(u0@Wx + u1@W1o + u2@W2o + bout)
The affine commutes through Agg and the dense transforms:
  with M_l = dinv*z_l (affine-free bf16 table), uM = Agg-of-M,
  W^T(a*uM) = (a*W)^T uM   and the bv part becomes a rank-1 term
  (W^T bv) (x) w,  w_i = dinv_i*sum_in dinv_j + dinv_i^2 (host-computed).
So each layer AllGathers M (built inside the previous layer's epilogue), the
a/bv folds ride the per-boundary weight prep, and the 1KB stats AllReduce
overlaps the AllGather.

Self-loops are not gathered: each tile adds one identity-selection matmul on
its own contiguous M rows (DMA'd back from gloc).

Sharding: 8 cores x 12800 nodes (12500 real + 300 pad), table row =
core*12800 + local.  One AllGather per layer (a hard barrier before the
layer's gathers -- CC and gather DMA contend destructively when overlapped).
Gather windows = 25600-row 2-core groups (int16).  dma_gather calls go
round-robin over 4 SWDGE queues (4 Q7 core-pairs generate descriptors in
parallel); per-call indices live only in the issuing queue's 32-partition
band.  Aggregation is tile-major: all of a tile's chunks accumulate in one
PSUM tile via selection-matrix matmuls, epilogue follows immediately.
"""
import sys

for p in ("/opt/trn_rl_repo", "/root/.axon_site"):
    if p not in sys.path:
        sys.path.insert(0, p)

import numpy as np

N = 100_000
E = 1_600_000
S = 8
P = 128
REAL = 12_500
TILES = 104            # tiles per core (greedy-balanced node->tile assignment)
SH = TILES * P         # 13312
NPAD = S * SH
Q = 4                  # gather windows (2-core groups)
WIN = 2 * SH           # 25600 rows per window
F = 128
HID = 128
C = 64
BN_EPS = 1e-5
MAXCH = 8              # chunks per gather call (1024 idx: the SWDGE
                       # per-call limit; >1024 wedges the device)
SLOTW = 8 * MAXCH      # idx16 cols per piece slot
NQUEUES = 4


def _chunk_offsets(K2):
    J2 = np.zeros((TILES, Q), dtype=np.int64)
    off = 0
    for q in range(Q):
        for t in range(TILES):
            J2[t, q] = off
            off += int(K2[t, q])
    return J2


def _piece_list(K2):
    """Gather calls (first chunk, n chunks, window), window-major."""
    J2 = _chunk_offsets(K2)
    out = []
    for q in range(Q):
        a = int(J2[0, q])
        end = int(J2[TILES - 1, q] + K2[TILES - 1, q])
        i = 0
        while a < end:
            w = min(MAXCH, end - a)
            out.append((a, w, q, i))
            a += w
            i += 1
    return out


def _piece_queue_slots(K2):
    """piece first-chunk -> (queue, idx16 col slot).  queue = (window +
    within-window index) % NQUEUES so each tile's four window-pieces hit all
    four queues regardless of per-window piece counts."""
    pieces = _piece_list(K2)
    qs = {}
    ctr = [0] * NQUEUES
    for (a, w, q, i) in pieces:
        qn = (q + i) % NQUEUES
        qs[a] = (qn, ctr[qn])
        ctr[qn] += 1
    return qs, max(ctr)


# ---------------------------------------------------------------- host prep
def _balanced_slots(dst, src_q):
    """Greedy per-core node->tile assignment balancing per-window indegree:
    keeps every (core, tile, window) bucket <= 512 edges (K2 = 4 uniform).
    Returns slot[v] = local slot in owner core (0..SH-1)."""
    dq = np.zeros((N, Q), dtype=np.int32)
    np.add.at(dq, (dst, src_q), 1)
    slot = np.empty(N, dtype=np.int64)
    cap = REAL // TILES + 1
    for c in range(S):
        nodes = np.arange(c * REAL, (c + 1) * REAL)
        dv = dq[nodes]
        order = np.argsort(-dv.max(1), kind="stable")
        Ssum = np.zeros((TILES, Q), dtype=np.int64)
        cnt = np.zeros(TILES, dtype=np.int64)
        tile_as = np.empty(REAL, dtype=np.int64)
        for i in order:
            cost = (Ssum + dv[i]).max(1).astype(np.float64)
            cost[cnt >= cap] = 1e18
            j = int(np.argmin(cost))
            tile_as[i] = j
            Ssum[j] += dv[i]
            cnt[j] += 1
        # pack nodes of each tile into slots
        off = np.zeros(TILES, dtype=np.int64)
        for i in range(REAL):
            j = tile_as[i]
            slot[c * REAL + i] = j * P + off[j]
            off[j] += 1
    return slot


def _prep_edges(edge_index):
    src = edge_index[0].astype(np.int64)
    dst = edge_index[1].astype(np.int64)

    deg = np.bincount(dst, minlength=N).astype(np.float32) + 1.0  # + self loop

    src_core = src // REAL
    src_q = src_core // 2                            # window = 2-core group
    slot = _balanced_slots(dst, src_q)
    src_winrel = (src_core % 2) * SH + slot[src]     # row within window

    dst_core = dst // REAL
    dst_local = slot[dst]
    tile_of = dst_local // P
    bucket = (dst_core * TILES + tile_of) * Q + src_q
    NBUK = S * TILES * Q
    cnt = np.bincount(bucket, minlength=NBUK).reshape(S, TILES, Q)
    K2 = np.ceil(cnt / P).astype(np.int64).max(axis=0)  # [TILES, Q]
    NCH = int(K2.sum())
    J2 = _chunk_offsets(K2)

    order = np.argsort(bucket, kind="stable")
    sw_s = src_winrel[order]
    buk_s = bucket[order]
    dr_s = (dst_local % P)[order]
    breaks = np.searchsorted(buk_s, np.arange(NBUK + 1))

    idx_flat = np.zeros((S, 16, 8 * NCH), dtype=np.int16)
    dstrel = np.full((S, P, NCH), -1.0, dtype=np.float32)

    for c in range(S):
        for t in range(TILES):
            for q in range(Q):
                kt = int(K2[t, q])
                if kt == 0:
                    continue
                b = (c * TILES + t) * Q + q
                lo, hi = breaks[b], breaks[b + 1]
                n = hi - lo
                j0 = int(J2[t, q])
                ii = np.zeros(kt * P, dtype=np.int16)  # pad -> row 0 (finite)
                if n:
                    ii[:n] = sw_s[lo:hi].astype(np.int16)
                iw = ii.reshape(kt * 8, 16).T  # flat n -> [n%16, n//16]
                idx_flat[c, :, 8 * j0 : 8 * (j0 + kt)] = iw
                dl = np.full(kt * P, -1.0, dtype=np.float32)
                if n:
                    dl[:n] = dr_s[lo:hi].astype(np.float32)
                dstrel[c, :, j0 : j0 + kt] = dl.reshape(kt, P).T

    # banded layout: call k runs on queue k%4 whose Q7 pair reads partitions
    # [32*(k%4), 32*(k%4)+32); pack 4 calls per 64-col slot, x2 within band.
    qs, NSLOT = _piece_queue_slots(K2)
    idx16 = np.zeros((S, P, SLOTW * NSLOT), dtype=np.int16)
    for (a, w, q, i) in _piece_list(K2):
        qn, sl_ = qs[a]
        blk = idx_flat[:, :, 8 * a : 8 * (a + w)]  # [S, 16, 8w]
        idx16[
            :, 32 * qn : 32 * qn + 32, SLOTW * sl_ : SLOTW * sl_ + 8 * w
        ] = np.tile(blk, (1, 2, 1))

    dinv = 1.0 / np.sqrt(deg)
    svec = np.zeros(N, dtype=np.float64)
    np.add.at(svec, dst, dinv[src].astype(np.float64))
    wvec = (dinv * svec + dinv * dinv).astype(np.float32)
    return idx16, dstrel, K2, deg, wvec, slot


def _split_excess_waits(nc, mybir, bass_rust, max_waits=1):
    ctr = [0]
    for bbname, bbw in nc.bb_map.items():
        insts = bbw.bb.instructions
        i = 0
        while i < len(insts):
            inst = insts[i]
            si = getattr(inst, "sync_info", None)
            waits = list(si.on_wait) if si is not None else []
            if len(waits) > max_waits:
                extra = waits[:-max_waits]
                chunks = [
                    extra[j : j + max_waits]
                    for j in range(0, len(extra), max_waits)
                ]
                for chunk in chunks:
                    ctr[0] += 1
                    nop = mybir.InstNoOp(name=f"wsplit-{ctr[0]}", ins=[], outs=[])
                    nop.engine = inst.engine
                    nop.sync_info = bass_rust.SyncInfo(on_wait=chunk, on_update=[])
                    insts.insert(i, nop)
                    i += 1
                si.on_wait = waits[-max_waits:]
            i += 1


# ---------------------------------------------------------------- device program
def _build_program(K2, skip_wait_split=False):
    import concourse.bass as bass
    import concourse.tile as tile
    from concourse import bacc as bacc_mod
    from concourse import mybir
    import bass_rust

    dt = mybir.dt
    AF = mybir.ActivationFunctionType
    NCH = int(K2.sum())
    J2 = _chunk_offsets(K2)
    KMAX = int(K2.max())
    R0 = [int(J2[0, q]) for q in range(Q)]
    REND = [int(J2[TILES - 1, q] + K2[TILES - 1, q]) for q in range(Q)]

    nc = bacc_mod.Bacc(
        "TRN2", target_bir_lowering=False, debug=False, num_devices=S,
        num_swdge_queues=NQUEUES, dynamic_dma_scratch_size=49152,
    )

    def din(name, shape, dtype=dt.float32):
        return nc.dram_tensor(name, shape, dtype, kind="ExternalInput").ap()

    piece_qs, NSLOT = _piece_queue_slots(K2)

    m0_d = din("m0", [SH, F], dt.bfloat16)
    m0full_d = din("m0full", [NPAD, F], dt.bfloat16)
    idx_d = din("idx16", [P, SLOTW * NSLOT], dt.int16)
    dsr_d = din("dstrel", [P, NCH])
    deg_d = din("deg", [P, TILES])
    wrow_d = din("wrow", [1, SH])
    W1_d = din("W1", [F, HID])
    W2_d = din("W2", [HID, HID])
    Wx_d = din("Wx", [F, C])
    W1o_d = din("W1o", [HID, C])
    W2o_d = din("W2o", [HID, C])
    b1_d = din("b1", [HID, 1])
    b2_d = din("b2", [HID, 1])
    bo_d = din("bout", [C, 1])
    gam_d = din("gamma", [HID, 1])
    bet_d = din("beta", [HID, 1])
    out_d = nc.dram_tensor("out", [SH, C], dt.float32, kind="ExternalOutput").ap()

    groups = [list(range(S))]

    with tile.TileContext(nc) as tc:
        with (
            tc.tile_pool(name="const", bufs=1) as cpool,
            tc.tile_pool(name="dram", bufs=1, space="DRAM") as dpool,
            tc.tile_pool(name="gath", bufs=20) as gpool,
            tc.tile_pool(name="mloc", bufs=3) as mpool,
            tc.tile_pool(name="sel", bufs=8) as spool,
            tc.tile_pool(name="acc", bufs=4, space="PSUM") as apool,
            tc.tile_pool(name="tpsum", bufs=1, space="PSUM") as tppool,
            tc.tile_pool(name="zpsum", bufs=2, space="PSUM") as zppool,
            tc.tile_pool(name="work", bufs=4) as wpool,
            tc.tile_pool(name="setup", bufs=1) as stpool,
            tc.tile_pool(name="epi", bufs=8) as epool,
        ):
            # ---------------- constants ----------------
            iota_i = stpool.tile([P, KMAX * P], dt.int32, tag="s0")
            nc.gpsimd.iota(
                iota_i[:], pattern=[[0, KMAX], [1, P]], channel_multiplier=0
            )
            iota_f = cpool.tile([P, KMAX * P], dt.bfloat16)
            nc.vector.tensor_copy(iota_f[:], iota_i[:])
            idxs = cpool.tile([P, SLOTW * NSLOT], dt.int16)
            nc.sync.dma_start(idxs[:], idx_d[:])
            dsrf = stpool.tile([P, NCH], dt.float32, tag="s1")
            nc.sync.dma_start(dsrf[:], dsr_d[:])
            dsrs = cpool.tile([P, NCH], dt.bfloat16)
            nc.vector.tensor_copy(dsrs[:], dsrf[:])
            degc = cpool.tile([P, TILES], dt.float32)
            nc.sync.dma_start(degc[:], deg_d[:])
            dinv_col = cpool.tile([P, TILES], dt.float32)
            nc.scalar.activation(dinv_col[:], degc[:], AF.Sqrt)
            nc.vector.reciprocal(dinv_col[:], dinv_col[:])

            ident = cpool.tile([P, P], dt.float32)
            ii = cpool.tile([P, P], dt.int32)
            nc.gpsimd.iota(ii[:], pattern=[[1, P]], channel_multiplier=0)
            iprel = cpool.tile([P, P], dt.int32)
            nc.gpsimd.iota(iprel[:], pattern=[[0, P]], channel_multiplier=1)
            nc.vector.tensor_tensor(
                ident[:], ii[:], iprel[:], op=mybir.AluOpType.is_equal
            )
            identb = cpool.tile([P, P], dt.bfloat16)
            nc.vector.tensor_copy(identb[:], ident[:])
            dgrid = cpool.tile([P, SH], dt.float32)
            for t in range(TILES):
                pt = tppool.tile([P, P], dt.float32, space="PSUM", tag="tp")
                nc.tensor.transpose(
                    out=pt[:],
                    in_=dinv_col[:, t : t + 1].to_broadcast([P, P]),
                    identity=ident[:],
                )
                nc.scalar.activation(dgrid[:, t * P : (t + 1) * P], pt[:], AF.Copy)

            def cload(name, dram, sh):
                t_ = cpool.tile(sh, dt.float32, tag=name)
                nc.sync.dma_start(t_[:], dram[:])
                return t_

            w1f = cload("w1f", W1_d, [F, HID])
            w2f = cload("w2f", W2_d, [HID, HID])
            wxf = cload("wxf", Wx_d, [F, C])
            w1of = cload("w1of", W1o_d, [HID, C])
            w2of = cload("w2of", W2o_d, [HID, C])
            b1c = cload("b1c", b1_d, [HID, 1])
            b2c = cload("b2c", b2_d, [HID, 1])
            boc = cload("boc", bo_d, [C, 1])
            gamc = cload("gamc", gam_d, [HID, 1])
            betc = cload("betc", bet_d, [HID, 1])

            # boundary-folded weights and rank-1 rows
            w2fa = cpool.tile([HID, HID], dt.float32)   # a1*W2
            w1ofa = cpool.tile([HID, C], dt.float32)    # a1*W1o
            w2ofa = cpool.tile([HID, C], dt.float32)    # a2*W2o
            czrow = cpool.tile([1, HID], dt.float32)    # bv1^T W2
            chrow = cpool.tile([1, C], dt.float32)      # bv1^T W1o + bv2^T W2o
            c1orow = cpool.tile([1, C], dt.float32)
            aff = [cpool.tile([HID, 2], dt.float32, tag=f"aff{l}",
                              name=f"aff{l}") for l in range(2)]

            zeroc = cpool.tile([HID, 1], dt.float32)
            nc.gpsimd.memset(zeroc[:], 0.0)
            epscc = cpool.tile([HID, 1], dt.float32)
            nc.gpsimd.memset(epscc[:], BN_EPS)
            statbuf = cpool.tile([HID, TILES], dt.float32, tag="stb1")
            statbuf2 = cpool.tile([HID, TILES], dt.float32, tag="stb2")

            gloc = [None] + [
                dpool.tile([SH, F], dt.bfloat16, name=f"gloc{l}",
                           tag=f"gloc{l}") for l in range(1, 3)
            ]
            # Layer 0's M table (dinv*x) is host-known, so the host ships the
            # FULL replicated table and layer 0 needs no AllGather at all:
            # gathers read m0full_d (IO reads are fine for DMA, just not for
            # collectives) and the self-loop reads the per-core m0_d.
            gsrc = [m0_d, gloc[1], gloc[2]]
            gfull = [None] + [
                dpool.tile([NPAD, F], dt.bfloat16, name=f"gfull{l}",
                           tag=f"gfull{l}", addr_space="Shared")
                for l in range(1, 3)
            ]
            gatherfull = [m0full_d, gfull[1], gfull[2]]
            u_dram = [dpool.tile([P, SH], dt.float32, name=f"u{l}",
                                 tag=f"u{l}") for l in range(2)]

            nidx_regs = {
                w: nc.gpsimd.to_reg(w * P) for w in range(1, MAXCH + 1)
            }

            # ---------------- helpers ----------------
            def emit_M_tile(l, Mf, t):
                """Transpose M tile (bf16 feature-major) -> gloc[l] rows."""
                ptp = tppool.tile([P, P], dt.bfloat16, space="PSUM", tag="tpb")
                nc.tensor.transpose(out=ptp, in_=Mf, identity=identb[:])
                rows = epool.tile([P, P], dt.bfloat16, tag="rows")
                nc.scalar.activation(rows[:], ptp[:], AF.Copy)
                nc.sync.dma_start(gloc[l][t * P : (t + 1) * P, :], rows[:])

            def launch_ag(l):
                nc.gpsimd.collective_compute(
                    "AllGather",
                    mybir.AluOpType.bypass,
                    replica_groups=groups,
                    ins=[gsrc[l][:]],
                    outs=[gfull[l][:]],
                )

            def aggregate_tile(l, t, pieces):
                def piece_for(j, q):
                    key = (q, (j - R0[q]) // MAXCH)
                    if key not in pieces:
                        a = R0[q] + key[1] * MAXCH
                        w = min(MAXCH, REND[q] - a)
                        qn, slot = piece_qs[a]
                        g = gpool.tile(
                            [P, MAXCH * F], dt.bfloat16, tag="g",
                            name=f"g{l}_{q}_{key[1]}",
                        )
                        nc.gpsimd.dma_gather(
                            out_ap=g[:, : w * F].rearrange(
                                "p (k f) -> p k f", k=w
                            ),
                            in_ap=gatherfull[l][
                                q * WIN : (q + 1) * WIN, :
                            ],
                            idxs_ap=idxs[:, SLOTW * slot : SLOTW * slot + 8 * w],
                            num_idxs=w * P,
                            num_idxs_reg=nidx_regs[w],
                            elem_size=F,
                            queue_num=qn,
                        )
                        pieces[key] = (g, a)
                    return pieces[key]

                cols = slice(t * P, (t + 1) * P)
                kt = int(K2[t].sum()) + 1            # +1: self-loop matmul
                acc = apool.tile([F, P], dt.float32, space="PSUM", tag="acc")
                mloc = mpool.tile([P, F], dt.bfloat16, tag="mloc")
                nc.scalar.dma_start(mloc[:], gsrc[l][t * P : (t + 1) * P, :])
                nc.tensor.matmul(
                    out=acc[:, :], lhsT=mloc[:], rhs=identb[:],
                    start=True, stop=(kt == 1),
                )
                mm = 1
                for q in range(Q):
                    kr = int(K2[t, q])
                    if kr == 0:
                        continue
                    j0 = int(J2[t, q])
                    st_ = spool.tile([P, KMAX * P], dt.bfloat16, tag="s")
                    nc.vector.tensor_tensor(
                        st_[:, : kr * P].rearrange("p (g c) -> p g c", g=kr),
                        dsrs[:, j0 : j0 + kr].to_broadcast([P, kr, P]),
                        iota_f[:, : kr * P].rearrange("p (g c) -> p g c", g=kr),
                        op=mybir.AluOpType.is_equal,
                    )
                    for k in range(kr):
                        g, a = piece_for(j0 + k, q)
                        o = j0 + k - a
                        nc.tensor.matmul(
                            out=acc[:, :],
                            lhsT=g[:, o * F : (o + 1) * F],
                            rhs=st_[:, k * P : (k + 1) * P],
                            start=False,
                            stop=(mm == kt - 1),
                        )
                        mm += 1

                # epilogue: uM = acc * dinv_dst (f32)
                uM = epool.tile([P, P], dt.float32, tag="uM")
                nc.vector.tensor_tensor(
                    uM[:], acc[:, :], dgrid[:, cols], op=mybir.AluOpType.mult
                )
                if l < 2:
                    nc.sync.dma_start(u_dram[l][:, cols], uM[:])
                    zp = zppool.tile([HID, P], dt.float32, space="PSUM",
                                     tag="zp")
                    if l == 0:
                        nc.tensor.matmul(
                            out=zp[:], lhsT=w1f[:], rhs=uM[:],
                            start=True, stop=True,
                        )
                        zbias = b1c
                    else:
                        nc.tensor.matmul(
                            out=zp[:], lhsT=w2fa[:], rhs=uM[:],
                            start=True, stop=False,
                        )
                        wrt = epool.tile([1, P], dt.float32, tag="wrt")
                        nc.sync.dma_start(wrt[:], wrow_d[0:1, cols])
                        nc.tensor.matmul(
                            out=zp[:], lhsT=czrow[:], rhs=wrt[:],
                            start=False, stop=True,
                        )
                        zbias = b2c
                    zt = epool.tile([HID, P], dt.float32, tag="zt")
                    nc.scalar.activation(
                        zt[:], zp[:], AF.Relu, bias=zbias[:, 0:1],
                        accum_out=statbuf[:, t : t + 1],
                    )
                    sq = epool.tile([HID, P], dt.float32, tag="sq")
                    nc.scalar.activation(
                        sq[:], zt[:], AF.Square,
                        accum_out=statbuf2[:, t : t + 1],
                    )
                    Mf = epool.tile([P, P], dt.bfloat16, tag="Mf")
                    nc.vector.tensor_tensor(
                        Mf[:], zt[:], dgrid[:, cols], op=mybir.AluOpType.mult
                    )
                    emit_M_tile(l + 1, Mf[:], t)
                else:
                    u0t = epool.tile([P, P], dt.float32, tag="u0t")
                    nc.scalar.dma_start(u0t[:], u_dram[0][:, cols])
                    u1t = epool.tile([P, P], dt.float32, tag="u1t")
                    nc.scalar.dma_start(u1t[:], u_dram[1][:, cols])
                    wrt = epool.tile([1, P], dt.float32, tag="wrt")
                    nc.sync.dma_start(wrt[:], wrow_d[0:1, cols])
                    zp = zppool.tile([C, P], dt.float32, space="PSUM", tag="zp")
                    nc.tensor.matmul(
                        out=zp[:], lhsT=wxf[:], rhs=u0t[:],
                        start=True, stop=False,
                    )
                    nc.tensor.matmul(
                        out=zp[:], lhsT=w1ofa[:], rhs=u1t[:],
                        start=False, stop=False,
                    )
                    nc.tensor.matmul(
                        out=zp[:], lhsT=w2ofa[:], rhs=uM[:],
                        start=False, stop=False,
                    )
                    nc.tensor.matmul(
                        out=zp[:], lhsT=chrow[:], rhs=wrt[:],
                        start=False, stop=True,
                    )
                    z3 = epool.tile([C, P], dt.float32, tag="z3")
                    nc.scalar.activation(
                        z3[:], zp[:], AF.Relu, bias=boc[:, 0:1]
                    )
                    ptp = tppool.tile([P, C], dt.float32, space="PSUM", tag="tp")
                    nc.tensor.transpose(
                        out=ptp[:], in_=z3[:], identity=ident[:C, :C]
                    )
                    onm = epool.tile([P, C], dt.float32, tag="onm")
                    nc.scalar.activation(onm[:], ptp[:], AF.Copy)
                    nc.sync.dma_start(out_d[t * P : (t + 1) * P, :], onm[:])

            def boundary_ar(l):
                """Launch the 1KB stats AllReduce (before the AllGather so it
                clears the in-order CC queue first and the folds overlap the
                AllGather)."""
                stl = dpool.tile([HID, 2], dt.float32, tag=f"stl{l}")
                sts = wpool.tile([HID, 2], dt.float32, tag="w2")
                nc.vector.reduce_sum(
                    sts[:, 0:1], statbuf[:], axis=mybir.AxisListType.X
                )
                nc.vector.reduce_sum(
                    sts[:, 1:2], statbuf2[:], axis=mybir.AxisListType.X
                )
                nc.sync.dma_start(stl[:], sts[:])
                star = dpool.tile([HID, 2], dt.float32, tag=f"star{l}")
                nc.gpsimd.collective_compute(
                    "AllReduce",
                    mybir.AluOpType.add,
                    replica_groups=groups,
                    ins=[stl[:]],
                    outs=[star[:]],
                )
                return star

            def boundary(l, star):
                """Stats -> a, bv; fold a into next weights; bv -> c-rows.
                Runs on gpsimd/scalar only: any op here waits on the
                AllReduce, and an AR-gated op in the DVE stream would
                head-of-line block the next layer's selection builds for the
                whole collective barrier."""
                zbias = b1c if l == 0 else b2c
                af = aff[l]
                stg = wpool.tile([HID, 2], dt.float32, tag="w2")
                nc.sync.dma_start(stg[:], star[:])
                rb = wpool.tile([HID, 2], dt.float32, tag="w0")
                nc.scalar.activation(
                    rb[:, 0:1], zeroc[:], AF.Relu, bias=zbias[:, 0:1]
                )
                nc.scalar.activation(rb[:, 1:2], rb[:, 0:1], AF.Square)
                corr = wpool.tile([HID, 2], dt.float32, tag="w1")
                nc.gpsimd.tensor_scalar(
                    out=corr[:], in0=rb[:], scalar1=-float(NPAD - N),
                    scalar2=None, op0=mybir.AluOpType.mult,
                )
                nc.gpsimd.tensor_add(stg[:], stg[:], corr[:])
                mv = wpool.tile([HID, 2], dt.float32, tag="w3")
                nc.gpsimd.tensor_scalar(
                    out=mv[:], in0=stg[:], scalar1=1.0 / N, scalar2=None,
                    op0=mybir.AluOpType.mult,
                )
                m2 = wpool.tile([HID, 1], dt.float32, tag="w0")
                nc.gpsimd.tensor_tensor(
                    m2[:], mv[:, 0:1], mv[:, 0:1], op=mybir.AluOpType.mult
                )
                var = wpool.tile([HID, 1], dt.float32, tag="w1")
                nc.gpsimd.tensor_sub(var[:], mv[:, 1:2], m2[:])
                # 1/sqrt(var+eps) = exp(-0.5*ln(var+eps)): keeps the whole
                # fold chain off the DVE stream (vector.reciprocal here would
                # head-of-line block the next layer's selection builds).
                lnv = wpool.tile([HID, 1], dt.float32, tag="w3")
                nc.scalar.activation(
                    lnv[:], var[:], AF.Ln, bias=epscc[:, 0:1]
                )
                nc.gpsimd.tensor_scalar(
                    out=lnv[:], in0=lnv[:], scalar1=-0.5, scalar2=None,
                    op0=mybir.AluOpType.mult,
                )
                sd = wpool.tile([HID, 1], dt.float32, tag="w1")
                nc.scalar.activation(sd[:], lnv[:], AF.Exp)
                nc.gpsimd.tensor_tensor(
                    af[:, 0:1], gamc[:], sd[:], op=mybir.AluOpType.mult
                )
                am = wpool.tile([HID, 1], dt.float32, tag="w0")
                nc.gpsimd.tensor_tensor(
                    am[:], af[:, 0:1], mv[:, 0:1], op=mybir.AluOpType.mult
                )
                nc.gpsimd.tensor_sub(af[:, 1:2], betc[:], am[:])

                if l == 0:
                    nc.gpsimd.tensor_scalar(
                        out=w2fa[:], in0=w2f[:], scalar1=af[:, 0:1],
                        scalar2=None, op0=mybir.AluOpType.mult,
                    )
                    nc.gpsimd.tensor_scalar(
                        out=w1ofa[:], in0=w1of[:], scalar1=af[:, 0:1],
                        scalar2=None, op0=mybir.AluOpType.mult,
                    )
                    czp = zppool.tile([1, HID], dt.float32, space="PSUM",
                                      tag="zp")
                    nc.tensor.matmul(
                        out=czp[:], lhsT=af[:, 1:2], rhs=w2f[:],
                        start=True, stop=True,
                    )
                    nc.scalar.activation(czrow[:], czp[:], AF.Copy)
                    c1p = zppool.tile([1, C], dt.float32, space="PSUM",
                                      tag="zp")
                    nc.tensor.matmul(
                        out=c1p[:], lhsT=af[:, 1:2], rhs=w1of[:],
                        start=True, stop=True,
                    )
                    nc.scalar.activation(c1orow[:], c1p[:], AF.Copy)
                else:
                    nc.gpsimd.tensor_scalar(
                        out=w2ofa[:], in0=w2of[:], scalar1=af[:, 0:1],
                        scalar2=None, op0=mybir.AluOpType.mult,
                    )
                    c2p = zppool.tile([1, C], dt.float32, space="PSUM",
                                      tag="zp")
                    nc.tensor.matmul(
                        out=c2p[:], lhsT=af[:, 1:2], rhs=w2of[:],
                        start=True, stop=True,
                    )
                    c2s = wpool.tile([1, C], dt.float32, tag="w0")
                    nc.scalar.activation(c2s[:], c2p[:], AF.Copy)
                    nc.gpsimd.tensor_add(chrow[:], c1orow[:], c2s[:])

            # ---------------- main schedule ----------------
            for l in range(3):
                pieces = {}
                for t in range(TILES):
                    aggregate_tile(l, t, pieces)
                if l < 2:
                    star = boundary_ar(l)
                    launch_ag(l + 1)
                    boundary(l, star)

    from concourse import mybir as _mybir

    nc.compile()
    if not skip_wait_split:
        _split_excess_waits(nc, _mybir, bass_rust, max_waits=1)
    return nc


def make_in_maps(x, edge_index, W1, b1, W2, b2, Wout, bout, gamma, beta):
    import ml_dtypes

    x = np.asarray(x, dtype=np.float32)
    edge_index = np.asarray(edge_index)
    idx16, dstrel, K2, deg, wvec, slot = _prep_edges(edge_index)

    dinv = 1.0 / np.sqrt(deg)
    m0p = np.zeros((S, SH, F), dtype=ml_dtypes.bfloat16)
    deg_cols = np.zeros((S, P, TILES), dtype=np.float32)
    wrows = np.zeros((S, 1, SH), dtype=np.float32)
    for c in range(S):
        sl = slot[c * REAL : (c + 1) * REAL]
        degp_c = np.full(SH, 1.0e30, dtype=np.float32)
        degp_c[sl] = deg[c * REAL : (c + 1) * REAL]
        deg_cols[c] = degp_c.reshape(TILES, P).T
        wrows[c, 0, sl] = wvec[c * REAL : (c + 1) * REAL]
        m0p[c][sl] = (
            x[c * REAL : (c + 1) * REAL]
            * dinv[c * REAL : (c + 1) * REAL, None]
        ).astype(ml_dtypes.bfloat16)
    m0full = np.ascontiguousarray(m0p.reshape(NPAD, F))

    W1 = np.asarray(W1, np.float32)
    W2 = np.asarray(W2, np.float32)
    Wout = np.asarray(Wout, np.float32)

    in_maps = []
    for c in range(S):
        in_maps.append(
            {
                "m0": np.ascontiguousarray(m0p[c]),
                "m0full": m0full,
                "idx16": idx16[c],
                "dstrel": dstrel[c],
                "deg": np.ascontiguousarray(deg_cols[c]),
                "wrow": np.ascontiguousarray(wrows[c]),
                "W1": W1,
                "W2": W2,
                "Wx": np.ascontiguousarray(Wout[0:F]),
                "W1o": np.ascontiguousarray(Wout[F : F + HID]),
                "W2o": np.ascontiguousarray(Wout[F + HID :]),
                "b1": np.asarray(b1, np.float32).reshape(-1, 1),
                "b2": np.asarray(b2, np.float32).reshape(-1, 1),
                "bout": np.asarray(bout, np.float32).reshape(-1, 1),
                "gamma": np.asarray(gamma, np.float32).reshape(-1, 1),
                "beta": np.asarray(beta, np.float32).reshape(-1, 1),
            }
        )
    return in_maps, K2, slot


_CACHE = {}
LAST_RESULT = None


def kernel(x, edge_index, W1, b1, W2, b2, Wout, bout, gamma, beta):
    global LAST_RESULT
    import os
    from concourse.bass_utils import run_bass_kernel_spmd

    in_maps, K2, slot = make_in_maps(
        x, edge_index, W1, b1, W2, b2, Wout, bout, gamma, beta
    )
    key = tuple(K2.ravel().tolist())
    if key not in _CACHE:
        _CACHE[key] = _build_program(K2)
    nc = _CACHE[key]

    trace = os.environ.get("GNN_TRACE", "") == "1"
    tmpdir = os.environ.get("GNN_TMPDIR") or None
    if tmpdir:
        os.makedirs(tmpdir, exist_ok=True)
    res = run_bass_kernel_spmd(
        nc, in_maps, list(range(S)), trace=trace, tmpdir=tmpdir
    )
    LAST_RESULT = res
    out = np.empty((N, C), dtype=np.float32)
    for c in range(S):
        sl = slot[c * REAL : (c + 1) * REAL]
        out[c * REAL : (c + 1) * REAL] = res.results[c]["out"][sl]
    return out



# revision 46
# speedup vs baseline: 1.0031x; 1.0031x over previous
"""3-layer GCN on 8 Trainium2 NeuronCores — aggregate-then-transform, v9.

Math (verified vs reference in f64):
  u_l = Agg(h_l),  Agg(v)_i = dinv_i*(sum_{j->i} dinv_j v_j) + dinv_i^2 v_i
  z1 = relu(u0@W1 + b1);  h_l = a_l*z_l + bv_l  (BatchNorm folded)
  z2 = relu(u1@W2 + b2);  out = relu(u0@Wx + u1@W1o + u2@W2o + bout)
The affine commutes through Agg and the dense transforms:
  with M_l = dinv*z_l (affine-free bf16 table), uM = Agg-of-M,
  W^T(a*uM) = (a*W)^T uM   and the bv part becomes a rank-1 term
  (W^T bv) (x) w,  w_i = dinv_i*sum_in dinv_j + dinv_i^2 (host-computed).
Layers 1/2 AllGather M (built inside the previous layer's epilogue) into a
Shared-address-space table (~2x faster than Local CC output).  Layer 0's
table M0 = dinv*x is host-known, so the host ships both the per-core rows
(m0, for the self-loop loads) and the FULL replicated table (m0full, read
directly by the gathers): layer 0 needs no collective at all.  The 1KB stats
AllReduce launches just before each AllGather; boundary folds then ride
under the collective.

Self-loops are not gathered: each tile adds one identity-selection matmul on
its own contiguous M rows (loaded on the scalar engine's HWDGE queue, like
the other latency-critical per-tile loads, to keep the sync queue for
stores).

Sharding: 8 cores x 12800 nodes (12500 real + 300 pad), table row =
core*12800 + local.  One AllGather per layer boundary (a hard barrier before
the layer's gathers -- CC and gather DMA contend destructively when
overlapped: NRT collectives preempt SWDGE traffic on the shared DMA
engines; chunked/overlapped CC measured 23% SLOWER end-to-end).  Gather
windows = 25600-row 2-core groups (int16).  dma_gather calls go round-robin
over 4 SWDGE queues (the ucode max; 1024 idx per call is the per-call max,
larger calls wedge the device); per-call indices live only in the issuing
queue's 32-partition band.  Aggregation is tile-major: all of a tile's
chunks accumulate in one PSUM tile via selection-matrix matmuls, epilogue
follows immediately.  fp8 tables were tried and fail the 2e-2 absmax gate
(6e-2): e4m3's worst-case 6% element error survives the 17-edge averaging.
"""
import sys

for p in ("/opt/trn_rl_repo", "/root/.axon_site"):
    if p not in sys.path:
        sys.path.insert(0, p)

import numpy as np

N = 100_000
E = 1_600_000
S = 8
P = 128
REAL = 12_500
TILES = 104            # tiles per core (greedy-balanced node->tile assignment)
SH = TILES * P         # 13312
NPAD = S * SH
Q = 4                  # gather windows (2-core groups)
WIN = 2 * SH           # 25600 rows per window
F = 128
HID = 128
C = 64
BN_EPS = 1e-5
MAXCH = 8              # chunks per gather call (1024 idx: the SWDGE
                       # per-call limit; >1024 wedges the device)
SLOTW = 8 * MAXCH      # idx16 cols per piece slot
NQUEUES = 4


def _chunk_offsets(K2):
    J2 = np.zeros((TILES, Q), dtype=np.int64)
    off = 0
    for q in range(Q):
        for t in range(TILES):
            J2[t, q] = off
            off += int(K2[t, q])
    return J2


def _piece_list(K2):
    """Gather calls (first chunk, n chunks, window), window-major."""
    J2 = _chunk_offsets(K2)
    out = []
    for q in range(Q):
        a = int(J2[0, q])
        end = int(J2[TILES - 1, q] + K2[TILES - 1, q])
        i = 0
        while a < end:
            w = min(MAXCH, end - a)
            out.append((a, w, q, i))
            a += w
            i += 1
    return out


def _piece_queue_slots(K2):
    """piece first-chunk -> (queue, idx16 col slot).  queue = (window +
    within-window index) % NQUEUES so each tile's four window-pieces hit all
    four queues regardless of per-window piece counts."""
    pieces = _piece_list(K2)
    qs = {}
    ctr = [0] * NQUEUES
    for (a, w, q, i) in pieces:
        qn = (q + i) % NQUEUES
        qs[a] = (qn, ctr[qn])
        ctr[qn] += 1
    return qs, max(ctr)


# ---------------------------------------------------------------- host prep
def _balanced_slots(dst, src_q):
    """Greedy per-core node->tile assignment balancing per-window indegree:
    keeps every (core, tile, window) bucket <= 512 edges (K2 = 4 uniform).
    Returns slot[v] = local slot in owner core (0..SH-1)."""
    dq = np.zeros((N, Q), dtype=np.int32)
    np.add.at(dq, (dst, src_q), 1)
    slot = np.empty(N, dtype=np.int64)
    cap = REAL // TILES + 1
    for c in range(S):
        nodes = np.arange(c * REAL, (c + 1) * REAL)
        dv = dq[nodes]
        order = np.argsort(-dv.max(1), kind="stable")
        Ssum = np.zeros((TILES, Q), dtype=np.int64)
        cnt = np.zeros(TILES, dtype=np.int64)
        tile_as = np.empty(REAL, dtype=np.int64)
        for i in order:
            cost = (Ssum + dv[i]).max(1).astype(np.float64)
            cost[cnt >= cap] = 1e18
            j = int(np.argmin(cost))
            tile_as[i] = j
            Ssum[j] += dv[i]
            cnt[j] += 1
        # pack nodes of each tile into slots
        off = np.zeros(TILES, dtype=np.int64)
        for i in range(REAL):
            j = tile_as[i]
            slot[c * REAL + i] = j * P + off[j]
            off[j] += 1
    return slot


def _prep_edges(edge_index):
    src = edge_index[0].astype(np.int64)
    dst = edge_index[1].astype(np.int64)

    deg = np.bincount(dst, minlength=N).astype(np.float32) + 1.0  # + self loop

    src_core = src // REAL
    src_q = src_core // 2                            # window = 2-core group
    slot = _balanced_slots(dst, src_q)
    src_winrel = (src_core % 2) * SH + slot[src]     # row within window

    dst_core = dst // REAL
    dst_local = slot[dst]
    tile_of = dst_local // P
    bucket = (dst_core * TILES + tile_of) * Q + src_q
    NBUK = S * TILES * Q
    cnt = np.bincount(bucket, minlength=NBUK).reshape(S, TILES, Q)
    K2 = np.ceil(cnt / P).astype(np.int64).max(axis=0)  # [TILES, Q]
    NCH = int(K2.sum())
    J2 = _chunk_offsets(K2)

    order = np.argsort(bucket, kind="stable")
    sw_s = src_winrel[order]
    buk_s = bucket[order]
    dr_s = (dst_local % P)[order]
    breaks = np.searchsorted(buk_s, np.arange(NBUK + 1))

    idx_flat = np.zeros((S, 16, 8 * NCH), dtype=np.int16)
    dstrel = np.full((S, P, NCH), -1.0, dtype=np.float32)

    for c in range(S):
        for t in range(TILES):
            for q in range(Q):
                kt = int(K2[t, q])
                if kt == 0:
                    continue
                b = (c * TILES + t) * Q + q
                lo, hi = breaks[b], breaks[b + 1]
                n = hi - lo
                j0 = int(J2[t, q])
                ii = np.zeros(kt * P, dtype=np.int16)  # pad -> row 0 (finite)
                if n:
                    ii[:n] = sw_s[lo:hi].astype(np.int16)
                iw = ii.reshape(kt * 8, 16).T  # flat n -> [n%16, n//16]
                idx_flat[c, :, 8 * j0 : 8 * (j0 + kt)] = iw
                dl = np.full(kt * P, -1.0, dtype=np.float32)
                if n:
                    dl[:n] = dr_s[lo:hi].astype(np.float32)
                dstrel[c, :, j0 : j0 + kt] = dl.reshape(kt, P).T

    # banded layout: call k runs on queue k%4 whose Q7 pair reads partitions
    # [32*(k%4), 32*(k%4)+32); pack 4 calls per 64-col slot, x2 within band.
    qs, NSLOT = _piece_queue_slots(K2)
    idx16 = np.zeros((S, P, SLOTW * NSLOT), dtype=np.int16)
    for (a, w, q, i) in _piece_list(K2):
        qn, sl_ = qs[a]
        blk = idx_flat[:, :, 8 * a : 8 * (a + w)]  # [S, 16, 8w]
        idx16[
            :, 32 * qn : 32 * qn + 32, SLOTW * sl_ : SLOTW * sl_ + 8 * w
        ] = np.tile(blk, (1, 2, 1))

    dinv = 1.0 / np.sqrt(deg)
    svec = np.zeros(N, dtype=np.float64)
    np.add.at(svec, dst, dinv[src].astype(np.float64))
    wvec = (dinv * svec + dinv * dinv).astype(np.float32)
    return idx16, dstrel, K2, deg, wvec, slot


def _split_excess_waits(nc, mybir, bass_rust, max_waits=1):
    ctr = [0]
    for bbname, bbw in nc.bb_map.items():
        insts = bbw.bb.instructions
        i = 0
        while i < len(insts):
            inst = insts[i]
            si = getattr(inst, "sync_info", None)
            waits = list(si.on_wait) if si is not None else []
            if len(waits) > max_waits:
                extra = waits[:-max_waits]
                chunks = [
                    extra[j : j + max_waits]
                    for j in range(0, len(extra), max_waits)
                ]
                for chunk in chunks:
                    ctr[0] += 1
                    nop = mybir.InstNoOp(name=f"wsplit-{ctr[0]}", ins=[], outs=[])
                    nop.engine = inst.engine
                    nop.sync_info = bass_rust.SyncInfo(on_wait=chunk, on_update=[])
                    insts.insert(i, nop)
                    i += 1
                si.on_wait = waits[-max_waits:]
            i += 1


# ---------------------------------------------------------------- device program
def _build_program(K2, skip_wait_split=False):
    import concourse.bass as bass
    import concourse.tile as tile
    from concourse import bacc as bacc_mod
    from concourse import mybir
    import bass_rust

    dt = mybir.dt
    AF = mybir.ActivationFunctionType
    NCH = int(K2.sum())
    J2 = _chunk_offsets(K2)
    KMAX = int(K2.max())
    R0 = [int(J2[0, q]) for q in range(Q)]
    REND = [int(J2[TILES - 1, q] + K2[TILES - 1, q]) for q in range(Q)]

    nc = bacc_mod.Bacc(
        "TRN2", target_bir_lowering=False, debug=False, num_devices=S,
        num_swdge_queues=NQUEUES, dynamic_dma_scratch_size=49152,
    )

    def din(name, shape, dtype=dt.float32):
        return nc.dram_tensor(name, shape, dtype, kind="ExternalInput").ap()

    piece_qs, NSLOT = _piece_queue_slots(K2)

    m0_d = din("m0", [SH, F], dt.bfloat16)
    m0full_d = din("m0full", [NPAD, F], dt.bfloat16)
    idx_d = din("idx16", [P, SLOTW * NSLOT], dt.int16)
    dsr_d = din("dstrel", [P, NCH])
    deg_d = din("deg", [P, TILES])
    wrow_d = din("wrow", [1, SH])
    W1_d = din("W1", [F, HID])
    W2_d = din("W2", [HID, HID])
    Wx_d = din("Wx", [F, C])
    W1o_d = din("W1o", [HID, C])
    W2o_d = din("W2o", [HID, C])
    b1_d = din("b1", [HID, 1])
    b2_d = din("b2", [HID, 1])
    bo_d = din("bout", [C, 1])
    gam_d = din("gamma", [HID, 1])
    bet_d = din("beta", [HID, 1])
    out_d = nc.dram_tensor("out", [SH, C], dt.float32, kind="ExternalOutput").ap()

    groups = [list(range(S))]

    with tile.TileContext(nc) as tc:
        with (
            tc.tile_pool(name="const", bufs=1) as cpool,
            tc.tile_pool(name="dram", bufs=1, space="DRAM") as dpool,
            tc.tile_pool(name="gath", bufs=20) as gpool,
            tc.tile_pool(name="mloc", bufs=3) as mpool,
            tc.tile_pool(name="sel", bufs=8) as spool,
            tc.tile_pool(name="acc", bufs=4, space="PSUM") as apool,
            tc.tile_pool(name="tpsum", bufs=1, space="PSUM") as tppool,
            tc.tile_pool(name="zpsum", bufs=2, space="PSUM") as zppool,
            tc.tile_pool(name="work", bufs=4) as wpool,
            tc.tile_pool(name="setup", bufs=1) as stpool,
            tc.tile_pool(name="epi", bufs=8) as epool,
        ):
            # ---------------- constants ----------------
            iota_i = stpool.tile([P, KMAX * P], dt.int32, tag="s0")
            nc.gpsimd.iota(
                iota_i[:], pattern=[[0, KMAX], [1, P]], channel_multiplier=0
            )
            iota_f = cpool.tile([P, KMAX * P], dt.bfloat16)
            nc.vector.tensor_copy(iota_f[:], iota_i[:])
            idxs = cpool.tile([P, SLOTW * NSLOT], dt.int16)
            nc.sync.dma_start(idxs[:], idx_d[:])
            dsrf = stpool.tile([P, NCH], dt.float32, tag="s1")
            nc.sync.dma_start(dsrf[:], dsr_d[:])
            dsrs = cpool.tile([P, NCH], dt.bfloat16)
            nc.vector.tensor_copy(dsrs[:], dsrf[:])
            degc = cpool.tile([P, TILES], dt.float32)
            nc.sync.dma_start(degc[:], deg_d[:])
            dinv_col = cpool.tile([P, TILES], dt.float32)
            nc.scalar.activation(dinv_col[:], degc[:], AF.Sqrt)
            nc.vector.reciprocal(dinv_col[:], dinv_col[:])

            ident = cpool.tile([P, P], dt.float32)
            ii = cpool.tile([P, P], dt.int32)
            nc.gpsimd.iota(ii[:], pattern=[[1, P]], channel_multiplier=0)
            iprel = cpool.tile([P, P], dt.int32)
            nc.gpsimd.iota(iprel[:], pattern=[[0, P]], channel_multiplier=1)
            nc.vector.tensor_tensor(
                ident[:], ii[:], iprel[:], op=mybir.AluOpType.is_equal
            )
            identb = cpool.tile([P, P], dt.bfloat16)
            nc.vector.tensor_copy(identb[:], ident[:])
            dgrid = cpool.tile([P, SH], dt.float32)
            for t in range(TILES):
                pt = tppool.tile([P, P], dt.float32, space="PSUM", tag="tp")
                nc.tensor.transpose(
                    out=pt[:],
                    in_=dinv_col[:, t : t + 1].to_broadcast([P, P]),
                    identity=ident[:],
                )
                nc.scalar.activation(dgrid[:, t * P : (t + 1) * P], pt[:], AF.Copy)

            def cload(name, dram, sh):
                t_ = cpool.tile(sh, dt.float32, tag=name)
                nc.sync.dma_start(t_[:], dram[:])
                return t_

            w1f = cload("w1f", W1_d, [F, HID])
            w2f = cload("w2f", W2_d, [HID, HID])
            wxf = cload("wxf", Wx_d, [F, C])
            w1of = cload("w1of", W1o_d, [HID, C])
            w2of = cload("w2of", W2o_d, [HID, C])
            b1c = cload("b1c", b1_d, [HID, 1])
            b2c = cload("b2c", b2_d, [HID, 1])
            boc = cload("boc", bo_d, [C, 1])
            gamc = cload("gamc", gam_d, [HID, 1])
            betc = cload("betc", bet_d, [HID, 1])

            # boundary-folded weights and rank-1 rows
            w2fa = cpool.tile([HID, HID], dt.float32)   # a1*W2
            w1ofa = cpool.tile([HID, C], dt.float32)    # a1*W1o
            w2ofa = cpool.tile([HID, C], dt.float32)    # a2*W2o
            czrow = cpool.tile([1, HID], dt.float32)    # bv1^T W2
            chrow = cpool.tile([1, C], dt.float32)      # bv1^T W1o + bv2^T W2o
            c1orow = cpool.tile([1, C], dt.float32)
            aff = [cpool.tile([HID, 2], dt.float32, tag=f"aff{l}",
                              name=f"aff{l}") for l in range(2)]

            zeroc = cpool.tile([HID, 1], dt.float32)
            nc.gpsimd.memset(zeroc[:], 0.0)
            epscc = cpool.tile([HID, 1], dt.float32)
            nc.gpsimd.memset(epscc[:], BN_EPS)
            statbuf = cpool.tile([HID, TILES], dt.float32, tag="stb1")
            statbuf2 = cpool.tile([HID, TILES], dt.float32, tag="stb2")

            gloc = [None] + [
                dpool.tile([SH, F], dt.bfloat16, name=f"gloc{l}",
                           tag=f"gloc{l}") for l in range(1, 3)
            ]
            # Layer 0's M table (dinv*x) is host-known, so the host ships the
            # FULL replicated table and layer 0 needs no AllGather at all:
            # gathers read m0full_d (IO reads are fine for DMA, just not for
            # collectives) and the self-loop reads the per-core m0_d.
            gsrc = [m0_d, gloc[1], gloc[2]]
            gfull = [None] + [
                dpool.tile([NPAD, F], dt.bfloat16, name=f"gfull{l}",
                           tag=f"gfull{l}", addr_space="Shared")
                for l in range(1, 3)
            ]
            gatherfull = [m0full_d, gfull[1], gfull[2]]
            u_dram = [dpool.tile([P, SH], dt.float32, name=f"u{l}",
                                 tag=f"u{l}") for l in range(2)]

            nidx_regs = {
                w: nc.gpsimd.to_reg(w * P) for w in range(1, MAXCH + 1)
            }

            # ---------------- helpers ----------------
            def emit_M_tile(l, Mf, t):
                """Transpose M tile (bf16 feature-major) -> gloc[l] rows."""
                ptp = tppool.tile([P, P], dt.bfloat16, space="PSUM", tag="tpb")
                nc.tensor.transpose(out=ptp, in_=Mf, identity=identb[:])
                rows = epool.tile([P, P], dt.bfloat16, tag="rows")
                nc.scalar.activation(rows[:], ptp[:], AF.Copy)
                nc.sync.dma_start(gloc[l][t * P : (t + 1) * P, :], rows[:])

            def launch_ag(l):
                nc.gpsimd.collective_compute(
                    "AllGather",
                    mybir.AluOpType.bypass,
                    replica_groups=groups,
                    ins=[gsrc[l][:]],
                    outs=[gfull[l][:]],
                )

            def aggregate_tile(l, t, pieces):
                def piece_for(j, q):
                    key = (q, (j - R0[q]) // MAXCH)
                    if key not in pieces:
                        a = R0[q] + key[1] * MAXCH
                        w = min(MAXCH, REND[q] - a)
                        qn, slot = piece_qs[a]
                        g = gpool.tile(
                            [P, MAXCH * F], dt.bfloat16, tag="g",
                            name=f"g{l}_{q}_{key[1]}",
                        )
                        nc.gpsimd.dma_gather(
                            out_ap=g[:, : w * F].rearrange(
                                "p (k f) -> p k f", k=w
                            ),
                            in_ap=gatherfull[l][
                                q * WIN : (q + 1) * WIN, :
                            ],
                            idxs_ap=idxs[:, SLOTW * slot : SLOTW * slot + 8 * w],
                            num_idxs=w * P,
                            num_idxs_reg=nidx_regs[w],
                            elem_size=F,
                            queue_num=qn,
                        )
                        pieces[key] = (g, a)
                    return pieces[key]

                cols = slice(t * P, (t + 1) * P)
                kt = int(K2[t].sum()) + 1            # +1: self-loop matmul
                acc = apool.tile([F, P], dt.float32, space="PSUM", tag="acc")
                mloc = mpool.tile([P, F], dt.bfloat16, tag="mloc")
                nc.scalar.dma_start(mloc[:], gsrc[l][t * P : (t + 1) * P, :])
                nc.tensor.matmul(
                    out=acc[:, :], lhsT=mloc[:], rhs=identb[:],
                    start=True, stop=(kt == 1),
                )
                mm = 1
                for q in range(Q):
                    kr = int(K2[t, q])
                    if kr == 0:
                        continue
                    j0 = int(J2[t, q])
                    st_ = spool.tile([P, KMAX * P], dt.bfloat16, tag="s")
                    nc.vector.tensor_tensor(
                        st_[:, : kr * P].rearrange("p (g c) -> p g c", g=kr),
                        dsrs[:, j0 : j0 + kr].to_broadcast([P, kr, P]),
                        iota_f[:, : kr * P].rearrange("p (g c) -> p g c", g=kr),
                        op=mybir.AluOpType.is_equal,
                    )
                    for k in range(kr):
                        g, a = piece_for(j0 + k, q)
                        o = j0 + k - a
                        nc.tensor.matmul(
                            out=acc[:, :],
                            lhsT=g[:, o * F : (o + 1) * F],
                            rhs=st_[:, k * P : (k + 1) * P],
                            start=False,
                            stop=(mm == kt - 1),
                        )
                        mm += 1

                # epilogue: uM = acc * dinv_dst (f32)
                uM = epool.tile([P, P], dt.float32, tag="uM")
                nc.vector.tensor_tensor(
                    uM[:], acc[:, :], dgrid[:, cols], op=mybir.AluOpType.mult
                )
                if l < 2:
                    nc.sync.dma_start(u_dram[l][:, cols], uM[:])
                    zp = zppool.tile([HID, P], dt.float32, space="PSUM",
                                     tag="zp")
                    if l == 0:
                        nc.tensor.matmul(
                            out=zp[:], lhsT=w1f[:], rhs=uM[:],
                            start=True, stop=True,
                        )
                        zbias = b1c
                    else:
                        nc.tensor.matmul(
                            out=zp[:], lhsT=w2fa[:], rhs=uM[:],
                            start=True, stop=False,
                        )
                        wrt = epool.tile([1, P], dt.float32, tag="wrt")
                        nc.sync.dma_start(wrt[:], wrow_d[0:1, cols])
                        nc.tensor.matmul(
                            out=zp[:], lhsT=czrow[:], rhs=wrt[:],
                            start=False, stop=True,
                        )
                        zbias = b2c
                    zt = epool.tile([HID, P], dt.float32, tag="zt")
                    nc.scalar.activation(
                        zt[:], zp[:], AF.Relu, bias=zbias[:, 0:1],
                        accum_out=statbuf[:, t : t + 1],
                    )
                    sq = epool.tile([HID, P], dt.float32, tag="sq")
                    nc.scalar.activation(
                        sq[:], zt[:], AF.Square,
                        accum_out=statbuf2[:, t : t + 1],
                    )
                    Mf = epool.tile([P, P], dt.bfloat16, tag="Mf")
                    nc.vector.tensor_tensor(
                        Mf[:], zt[:], dgrid[:, cols], op=mybir.AluOpType.mult
                    )
                    emit_M_tile(l + 1, Mf[:], t)
                else:
                    u0t = epool.tile([P, P], dt.float32, tag="u0t")
                    nc.scalar.dma_start(u0t[:], u_dram[0][:, cols])
                    u1t = epool.tile([P, P], dt.float32, tag="u1t")
                    nc.scalar.dma_start(u1t[:], u_dram[1][:, cols])
                    wrt = epool.tile([1, P], dt.float32, tag="wrt")
                    nc.sync.dma_start(wrt[:], wrow_d[0:1, cols])
                    zp = zppool.tile([C, P], dt.float32, space="PSUM", tag="zp")
                    nc.tensor.matmul(
                        out=zp[:], lhsT=wxf[:], rhs=u0t[:],
                        start=True, stop=False,
                    )
                    nc.tensor.matmul(
                        out=zp[:], lhsT=w1ofa[:], rhs=u1t[:],
                        start=False, stop=False,
                    )
                    nc.tensor.matmul(
                        out=zp[:], lhsT=w2ofa[:], rhs=uM[:],
                        start=False, stop=False,
                    )
                    nc.tensor.matmul(
                        out=zp[:], lhsT=chrow[:], rhs=wrt[:],
                        start=False, stop=True,
                    )
                    z3 = epool.tile([C, P], dt.float32, tag="z3")
                    nc.scalar.activation(
                        z3[:], zp[:], AF.Relu, bias=boc[:, 0:1]
                    )
                    ptp = tppool.tile([P, C], dt.float32, space="PSUM", tag="tp")
                    nc.tensor.transpose(
                        out=ptp[:], in_=z3[:], identity=ident[:C, :C]
                    )
                    onm = epool.tile([P, C], dt.float32, tag="onm")
                    nc.scalar.activation(onm[:], ptp[:], AF.Copy)
                    nc.sync.dma_start(out_d[t * P : (t + 1) * P, :], onm[:])

            def boundary_ar(l):
                """Launch the 1KB stats AllReduce (before the AllGather so it
                clears the in-order CC queue first and the folds overlap the
                AllGather)."""
                stl = dpool.tile([HID, 2], dt.float32, tag=f"stl{l}")
                sts = wpool.tile([HID, 2], dt.float32, tag="w2")
                nc.vector.reduce_sum(
                    sts[:, 0:1], statbuf[:], axis=mybir.AxisListType.X
                )
                nc.vector.reduce_sum(
                    sts[:, 1:2], statbuf2[:], axis=mybir.AxisListType.X
                )
                nc.sync.dma_start(stl[:], sts[:])
                star = dpool.tile([HID, 2], dt.float32, tag=f"star{l}")
                nc.gpsimd.collective_compute(
                    "AllReduce",
                    mybir.AluOpType.add,
                    replica_groups=groups,
                    ins=[stl[:]],
                    outs=[star[:]],
                )
                return star

            def boundary(l, star):
                """Stats -> a, bv; fold a into next weights; bv -> c-rows.
                Runs on gpsimd/scalar only: any op here waits on the
                AllReduce, and an AR-gated op in the DVE stream would
                head-of-line block the next layer's selection builds for the
                whole collective barrier."""
                zbias = b1c if l == 0 else b2c
                af = aff[l]
                stg = wpool.tile([HID, 2], dt.float32, tag="w2")
                nc.sync.dma_start(stg[:], star[:])
                rb = wpool.tile([HID, 2], dt.float32, tag="w0")
                nc.scalar.activation(
                    rb[:, 0:1], zeroc[:], AF.Relu, bias=zbias[:, 0:1]
                )
                nc.scalar.activation(rb[:, 1:2], rb[:, 0:1], AF.Square)
                corr = wpool.tile([HID, 2], dt.float32, tag="w1")
                nc.gpsimd.tensor_scalar(
                    out=corr[:], in0=rb[:], scalar1=-float(NPAD - N),
                    scalar2=None, op0=mybir.AluOpType.mult,
                )
                nc.gpsimd.tensor_add(stg[:], stg[:], corr[:])
                mv = wpool.tile([HID, 2], dt.float32, tag="w3")
                nc.gpsimd.tensor_scalar(
                    out=mv[:], in0=stg[:], scalar1=1.0 / N, scalar2=None,
                    op0=mybir.AluOpType.mult,
                )
                m2 = wpool.tile([HID, 1], dt.float32, tag="w0")
                nc.gpsimd.tensor_tensor(
                    m2[:], mv[:, 0:1], mv[:, 0:1], op=mybir.AluOpType.mult
                )
                var = wpool.tile([HID, 1], dt.float32, tag="w1")
                nc.gpsimd.tensor_sub(var[:], mv[:, 1:2], m2[:])
                # 1/sqrt(var+eps) = exp(-0.5*ln(var+eps)): keeps the whole
                # fold chain off the DVE stream (vector.reciprocal here would
                # head-of-line block the next layer's selection builds).
                lnv = wpool.tile([HID, 1], dt.float32, tag="w3")
                nc.scalar.activation(
                    lnv[:], var[:], AF.Ln, bias=epscc[:, 0:1]
                )
                nc.gpsimd.tensor_scalar(
                    out=lnv[:], in0=lnv[:], scalar1=-0.5, scalar2=None,
                    op0=mybir.AluOpType.mult,
                )
                sd = wpool.tile([HID, 1], dt.float32, tag="w1")
                nc.scalar.activation(sd[:], lnv[:], AF.Exp)
                nc.gpsimd.tensor_tensor(
                    af[:, 0:1], gamc[:], sd[:], op=mybir.AluOpType.mult
                )
                am = wpool.tile([HID, 1], dt.float32, tag="w0")
                nc.gpsimd.tensor_tensor(
                    am[:], af[:, 0:1], mv[:, 0:1], op=mybir.AluOpType.mult
                )
                nc.gpsimd.tensor_sub(af[:, 1:2], betc[:], am[:])

                if l == 0:
                    nc.gpsimd.tensor_scalar(
                        out=w2fa[:], in0=w2f[:], scalar1=af[:, 0:1],
                        scalar2=None, op0=mybir.AluOpType.mult,
                    )
                    nc.gpsimd.tensor_scalar(
                        out=w1ofa[:], in0=w1of[:], scalar1=af[:, 0:1],
                        scalar2=None, op0=mybir.AluOpType.mult,
                    )
                    czp = zppool.tile([1, HID], dt.float32, space="PSUM",
                                      tag="zp")
                    nc.tensor.matmul(
                        out=czp[:], lhsT=af[:, 1:2], rhs=w2f[:],
                        start=True, stop=True,
                    )
                    nc.scalar.activation(czrow[:], czp[:], AF.Copy)
                    c1p = zppool.tile([1, C], dt.float32, space="PSUM",
                                      tag="zp")
                    nc.tensor.matmul(
                        out=c1p[:], lhsT=af[:, 1:2], rhs=w1of[:],
                        start=True, stop=True,
                    )
                    nc.scalar.activation(c1orow[:], c1p[:], AF.Copy)
                else:
                    nc.gpsimd.tensor_scalar(
                        out=w2ofa[:], in0=w2of[:], scalar1=af[:, 0:1],
                        scalar2=None, op0=mybir.AluOpType.mult,
                    )
                    c2p = zppool.tile([1, C], dt.float32, space="PSUM",
                                      tag="zp")
                    nc.tensor.matmul(
                        out=c2p[:], lhsT=af[:, 1:2], rhs=w2of[:],
                        start=True, stop=True,
                    )
                    c2s = wpool.tile([1, C], dt.float32, tag="w0")
                    nc.scalar.activation(c2s[:], c2p[:], AF.Copy)
                    nc.gpsimd.tensor_add(chrow[:], c1orow[:], c2s[:])

            # ---------------- main schedule ----------------
            for l in range(3):
                pieces = {}
                for t in range(TILES):
                    aggregate_tile(l, t, pieces)
                if l < 2:
                    star = boundary_ar(l)
                    launch_ag(l + 1)
                    boundary(l, star)

    from concourse import mybir as _mybir

    nc.compile()
    if not skip_wait_split:
        _split_excess_waits(nc, _mybir, bass_rust, max_waits=1)
    return nc


def make_in_maps(x, edge_index, W1, b1, W2, b2, Wout, bout, gamma, beta):
    import ml_dtypes

    x = np.asarray(x, dtype=np.float32)
    edge_index = np.asarray(edge_index)
    idx16, dstrel, K2, deg, wvec, slot = _prep_edges(edge_index)

    dinv = 1.0 / np.sqrt(deg)
    m0p = np.zeros((S, SH, F), dtype=ml_dtypes.bfloat16)
    deg_cols = np.zeros((S, P, TILES), dtype=np.float32)
    wrows = np.zeros((S, 1, SH), dtype=np.float32)
    for c in range(S):
        sl = slot[c * REAL : (c + 1) * REAL]
        degp_c = np.full(SH, 1.0e30, dtype=np.float32)
        degp_c[sl] = deg[c * REAL : (c + 1) * REAL]
        deg_cols[c] = degp_c.reshape(TILES, P).T
        wrows[c, 0, sl] = wvec[c * REAL : (c + 1) * REAL]
        m0p[c][sl] = (
            x[c * REAL : (c + 1) * REAL]
            * dinv[c * REAL : (c + 1) * REAL, None]
        ).astype(ml_dtypes.bfloat16)
    m0full = np.ascontiguousarray(m0p.reshape(NPAD, F))

    W1 = np.asarray(W1, np.float32)
    W2 = np.asarray(W2, np.float32)
    Wout = np.asarray(Wout, np.float32)

    in_maps = []
    for c in range(S):
        in_maps.append(
            {
                "m0": np.ascontiguousarray(m0p[c]),
                "m0full": m0full,
                "idx16": idx16[c],
                "dstrel": dstrel[c],
                "deg": np.ascontiguousarray(deg_cols[c]),
                "wrow": np.ascontiguousarray(wrows[c]),
                "W1": W1,
                "W2": W2,
                "Wx": np.ascontiguousarray(Wout[0:F]),
                "W1o": np.ascontiguousarray(Wout[F : F + HID]),
                "W2o": np.ascontiguousarray(Wout[F + HID :]),
                "b1": np.asarray(b1, np.float32).reshape(-1, 1),
                "b2": np.asarray(b2, np.float32).reshape(-1, 1),
                "bout": np.asarray(bout, np.float32).reshape(-1, 1),
                "gamma": np.asarray(gamma, np.float32).reshape(-1, 1),
                "beta": np.asarray(beta, np.float32).reshape(-1, 1),
            }
        )
    return in_maps, K2, slot


_CACHE = {}
LAST_RESULT = None


def kernel(x, edge_index, W1, b1, W2, b2, Wout, bout, gamma, beta):
    global LAST_RESULT
    import os
    from concourse.bass_utils import run_bass_kernel_spmd

    in_maps, K2, slot = make_in_maps(
        x, edge_index, W1, b1, W2, b2, Wout, bout, gamma, beta
    )
    key = tuple(K2.ravel().tolist())
    if key not in _CACHE:
        _CACHE[key] = _build_program(K2)
    nc = _CACHE[key]

    trace = os.environ.get("GNN_TRACE", "") == "1"
    tmpdir = os.environ.get("GNN_TMPDIR") or None
    if tmpdir:
        os.makedirs(tmpdir, exist_ok=True)
    res = run_bass_kernel_spmd(
        nc, in_maps, list(range(S)), trace=trace, tmpdir=tmpdir
    )
    LAST_RESULT = res
    out = np.empty((N, C), dtype=np.float32)
    for c in range(S):
        sl = slot[c * REAL : (c + 1) * REAL]
        out[c * REAL : (c + 1) * REAL] = res.results[c]["out"][sl]
    return out



# revision 47
# speedup vs baseline: 1.0519x; 1.0486x over previous
"""3-layer GCN on 8 Trainium2 NeuronCores — aggregate-then-transform, v9.

Math (verified vs reference in f64):
  u_l = Agg(h_l),  Agg(v)_i = dinv_i*(sum_{j->i} dinv_j v_j) + dinv_i^2 v_i
  z1 = relu(u0@W1 + b1);  h_l = a_l*z_l + bv_l  (BatchNorm folded)
  z2 = relu(u1@W2 + b2);  out = relu(u0@Wx + u1@W1o + u2@W2o + bout)
The affine commutes through Agg and the dense transforms:
  with M_l = dinv*z_l (affine-free bf16 table), uM = Agg-of-M,
  W^T(a*uM) = (a*W)^T uM   and the bv part becomes a rank-1 term
  (W^T bv) (x) w,  w_i = dinv_i*sum_in dinv_j + dinv_i^2 (host-computed).
Layers 1/2 AllGather M (built inside the previous layer's epilogue) into a
Shared-address-space table (~2x faster than Local CC output).  Layer 0's
table M0 = dinv*x is host-known, so the host ships both the per-core rows
(m0, for the self-loop loads) and the FULL replicated table (m0full, read
directly by the gathers): layer 0 needs no collective at all.  The 1KB stats
AllReduce launches just before each AllGather; boundary folds then ride
under the collective.

Self-loops are not gathered: each tile adds one identity-selection matmul on
its own contiguous M rows (loaded on the scalar engine's HWDGE queue, like
the other latency-critical per-tile loads, to keep the sync queue for
stores).

Sharding: 8 cores x 12800 nodes (12500 real + 300 pad), table row =
core*12800 + local.  One AllGather per layer boundary (a hard barrier before
the layer's gathers -- CC and gather DMA contend destructively when
overlapped: NRT collectives preempt SWDGE traffic on the shared DMA
engines; chunked/overlapped CC measured 23% SLOWER end-to-end).  Gather
windows = 25600-row 2-core groups (int16).  dma_gather calls go round-robin
over 4 SWDGE queues (the ucode max; 1024 idx per call is the per-call max,
larger calls wedge the device); per-call indices live only in the issuing
queue's 32-partition band.  Aggregation is tile-major: all of a tile's
chunks accumulate in one PSUM tile via selection-matrix matmuls, epilogue
follows immediately.  fp8 tables were tried and fail the 2e-2 absmax gate
(6e-2): e4m3's worst-case 6% element error survives the 17-edge averaging.
"""
import sys

for p in ("/opt/trn_rl_repo", "/root/.axon_site"):
    if p not in sys.path:
        sys.path.insert(0, p)

import numpy as np

N = 100_000
E = 1_600_000
S = 8
P = 128
REAL = 12_500
TILES = 100            # tiles per core (smallest with all buckets <= 512)
SH = TILES * P         # 13312
NPAD = S * SH
Q = 4                  # gather windows (2-core groups)
WIN = 2 * SH           # 25600 rows per window
F = 128
HID = 128
C = 64
BN_EPS = 1e-5
MAXCH = 8              # chunks per gather call (1024 idx: the SWDGE
                       # per-call limit; >1024 wedges the device)
SLOTW = 8 * MAXCH      # idx16 cols per piece slot
NQUEUES = 4


def _chunk_offsets(K2):
    J2 = np.zeros((TILES, Q), dtype=np.int64)
    off = 0
    for q in range(Q):
        for t in range(TILES):
            J2[t, q] = off
            off += int(K2[t, q])
    return J2


def _piece_list(K2):
    """Gather calls (first chunk, n chunks, window), window-major."""
    J2 = _chunk_offsets(K2)
    out = []
    for q in range(Q):
        a = int(J2[0, q])
        end = int(J2[TILES - 1, q] + K2[TILES - 1, q])
        i = 0
        while a < end:
            w = min(MAXCH, end - a)
            out.append((a, w, q, i))
            a += w
            i += 1
    return out


def _piece_queue_slots(K2):
    """piece first-chunk -> (queue, idx16 col slot).  queue = (window +
    within-window index) % NQUEUES so each tile's four window-pieces hit all
    four queues regardless of per-window piece counts."""
    pieces = _piece_list(K2)
    qs = {}
    ctr = [0] * NQUEUES
    for (a, w, q, i) in pieces:
        qn = (q + i) % NQUEUES
        qs[a] = (qn, ctr[qn])
        ctr[qn] += 1
    return qs, max(ctr)


# ---------------------------------------------------------------- host prep
def _balanced_slots(dst, src_q):
    """Greedy per-core node->tile assignment balancing per-window indegree:
    keeps every (core, tile, window) bucket <= 512 edges (K2 = 4 uniform).
    Returns slot[v] = local slot in owner core (0..SH-1)."""
    dq = np.zeros((N, Q), dtype=np.int32)
    np.add.at(dq, (dst, src_q), 1)
    slot = np.empty(N, dtype=np.int64)
    cap = REAL // TILES + 1
    for c in range(S):
        nodes = np.arange(c * REAL, (c + 1) * REAL)
        dv = dq[nodes]
        order = np.argsort(-dv.max(1), kind="stable")
        Ssum = np.zeros((TILES, Q), dtype=np.int64)
        cnt = np.zeros(TILES, dtype=np.int64)
        tile_as = np.empty(REAL, dtype=np.int64)
        for i in order:
            cost = (Ssum + dv[i]).max(1).astype(np.float64)
            cost[cnt >= cap] = 1e18
            j = int(np.argmin(cost))
            tile_as[i] = j
            Ssum[j] += dv[i]
            cnt[j] += 1
        # pack nodes of each tile into slots
        off = np.zeros(TILES, dtype=np.int64)
        for i in range(REAL):
            j = tile_as[i]
            slot[c * REAL + i] = j * P + off[j]
            off[j] += 1
    return slot


def _prep_edges(edge_index):
    src = edge_index[0].astype(np.int64)
    dst = edge_index[1].astype(np.int64)

    deg = np.bincount(dst, minlength=N).astype(np.float32) + 1.0  # + self loop

    src_core = src // REAL
    src_q = src_core // 2                            # window = 2-core group
    slot = _balanced_slots(dst, src_q)
    src_winrel = (src_core % 2) * SH + slot[src]     # row within window

    dst_core = dst // REAL
    dst_local = slot[dst]
    tile_of = dst_local // P
    bucket = (dst_core * TILES + tile_of) * Q + src_q
    NBUK = S * TILES * Q
    cnt = np.bincount(bucket, minlength=NBUK).reshape(S, TILES, Q)
    K2 = np.ceil(cnt / P).astype(np.int64).max(axis=0)  # [TILES, Q]
    NCH = int(K2.sum())
    J2 = _chunk_offsets(K2)

    order = np.argsort(bucket, kind="stable")
    sw_s = src_winrel[order]
    buk_s = bucket[order]
    dr_s = (dst_local % P)[order]
    breaks = np.searchsorted(buk_s, np.arange(NBUK + 1))

    idx_flat = np.zeros((S, 16, 8 * NCH), dtype=np.int16)
    dstrel = np.full((S, P, NCH), -1.0, dtype=np.float32)

    for c in range(S):
        for t in range(TILES):
            for q in range(Q):
                kt = int(K2[t, q])
                if kt == 0:
                    continue
                b = (c * TILES + t) * Q + q
                lo, hi = breaks[b], breaks[b + 1]
                n = hi - lo
                j0 = int(J2[t, q])
                ii = np.zeros(kt * P, dtype=np.int16)  # pad -> row 0 (finite)
                if n:
                    ii[:n] = sw_s[lo:hi].astype(np.int16)
                iw = ii.reshape(kt * 8, 16).T  # flat n -> [n%16, n//16]
                idx_flat[c, :, 8 * j0 : 8 * (j0 + kt)] = iw
                dl = np.full(kt * P, -1.0, dtype=np.float32)
                if n:
                    dl[:n] = dr_s[lo:hi].astype(np.float32)
                dstrel[c, :, j0 : j0 + kt] = dl.reshape(kt, P).T

    # banded layout: call k runs on queue k%4 whose Q7 pair reads partitions
    # [32*(k%4), 32*(k%4)+32); pack 4 calls per 64-col slot, x2 within band.
    qs, NSLOT = _piece_queue_slots(K2)
    idx16 = np.zeros((S, P, SLOTW * NSLOT), dtype=np.int16)
    for (a, w, q, i) in _piece_list(K2):
        qn, sl_ = qs[a]
        blk = idx_flat[:, :, 8 * a : 8 * (a + w)]  # [S, 16, 8w]
        idx16[
            :, 32 * qn : 32 * qn + 32, SLOTW * sl_ : SLOTW * sl_ + 8 * w
        ] = np.tile(blk, (1, 2, 1))

    dinv = 1.0 / np.sqrt(deg)
    svec = np.zeros(N, dtype=np.float64)
    np.add.at(svec, dst, dinv[src].astype(np.float64))
    wvec = (dinv * svec + dinv * dinv).astype(np.float32)
    return idx16, dstrel, K2, deg, wvec, slot


def _split_excess_waits(nc, mybir, bass_rust, max_waits=1):
    ctr = [0]
    for bbname, bbw in nc.bb_map.items():
        insts = bbw.bb.instructions
        i = 0
        while i < len(insts):
            inst = insts[i]
            si = getattr(inst, "sync_info", None)
            waits = list(si.on_wait) if si is not None else []
            if len(waits) > max_waits:
                extra = waits[:-max_waits]
                chunks = [
                    extra[j : j + max_waits]
                    for j in range(0, len(extra), max_waits)
                ]
                for chunk in chunks:
                    ctr[0] += 1
                    nop = mybir.InstNoOp(name=f"wsplit-{ctr[0]}", ins=[], outs=[])
                    nop.engine = inst.engine
                    nop.sync_info = bass_rust.SyncInfo(on_wait=chunk, on_update=[])
                    insts.insert(i, nop)
                    i += 1
                si.on_wait = waits[-max_waits:]
            i += 1


# ---------------------------------------------------------------- device program
def _build_program(K2, skip_wait_split=False):
    import concourse.bass as bass
    import concourse.tile as tile
    from concourse import bacc as bacc_mod
    from concourse import mybir
    import bass_rust

    dt = mybir.dt
    AF = mybir.ActivationFunctionType
    NCH = int(K2.sum())
    J2 = _chunk_offsets(K2)
    KMAX = int(K2.max())
    R0 = [int(J2[0, q]) for q in range(Q)]
    REND = [int(J2[TILES - 1, q] + K2[TILES - 1, q]) for q in range(Q)]

    nc = bacc_mod.Bacc(
        "TRN2", target_bir_lowering=False, debug=False, num_devices=S,
        num_swdge_queues=NQUEUES, dynamic_dma_scratch_size=49152,
    )

    def din(name, shape, dtype=dt.float32):
        return nc.dram_tensor(name, shape, dtype, kind="ExternalInput").ap()

    piece_qs, NSLOT = _piece_queue_slots(K2)

    m0_d = din("m0", [SH, F], dt.bfloat16)
    m0full_d = din("m0full", [NPAD, F], dt.bfloat16)
    idx_d = din("idx16", [P, SLOTW * NSLOT], dt.int16)
    dsr_d = din("dstrel", [P, NCH])
    deg_d = din("deg", [P, TILES])
    wrow_d = din("wrow", [1, SH])
    W1_d = din("W1", [F, HID])
    W2_d = din("W2", [HID, HID])
    Wx_d = din("Wx", [F, C])
    W1o_d = din("W1o", [HID, C])
    W2o_d = din("W2o", [HID, C])
    b1_d = din("b1", [HID, 1])
    b2_d = din("b2", [HID, 1])
    bo_d = din("bout", [C, 1])
    gam_d = din("gamma", [HID, 1])
    bet_d = din("beta", [HID, 1])
    out_d = nc.dram_tensor("out", [SH, C], dt.float32, kind="ExternalOutput").ap()

    groups = [list(range(S))]

    with tile.TileContext(nc) as tc:
        with (
            tc.tile_pool(name="const", bufs=1) as cpool,
            tc.tile_pool(name="dram", bufs=1, space="DRAM") as dpool,
            tc.tile_pool(name="gath", bufs=20) as gpool,
            tc.tile_pool(name="mloc", bufs=3) as mpool,
            tc.tile_pool(name="sel", bufs=8) as spool,
            tc.tile_pool(name="acc", bufs=4, space="PSUM") as apool,
            tc.tile_pool(name="tpsum", bufs=1, space="PSUM") as tppool,
            tc.tile_pool(name="zpsum", bufs=2, space="PSUM") as zppool,
            tc.tile_pool(name="work", bufs=4) as wpool,
            tc.tile_pool(name="setup", bufs=1) as stpool,
            tc.tile_pool(name="epi", bufs=8) as epool,
        ):
            # ---------------- constants ----------------
            iota_i = stpool.tile([P, KMAX * P], dt.int32, tag="s0")
            nc.gpsimd.iota(
                iota_i[:], pattern=[[0, KMAX], [1, P]], channel_multiplier=0
            )
            iota_f = cpool.tile([P, KMAX * P], dt.bfloat16)
            nc.vector.tensor_copy(iota_f[:], iota_i[:])
            idxs = cpool.tile([P, SLOTW * NSLOT], dt.int16)
            nc.sync.dma_start(idxs[:], idx_d[:])
            dsrf = stpool.tile([P, NCH], dt.float32, tag="s1")
            nc.sync.dma_start(dsrf[:], dsr_d[:])
            dsrs = cpool.tile([P, NCH], dt.bfloat16)
            nc.vector.tensor_copy(dsrs[:], dsrf[:])
            degc = cpool.tile([P, TILES], dt.float32)
            nc.sync.dma_start(degc[:], deg_d[:])
            dinv_col = cpool.tile([P, TILES], dt.float32)
            nc.scalar.activation(dinv_col[:], degc[:], AF.Sqrt)
            nc.vector.reciprocal(dinv_col[:], dinv_col[:])

            ident = cpool.tile([P, P], dt.float32)
            ii = cpool.tile([P, P], dt.int32)
            nc.gpsimd.iota(ii[:], pattern=[[1, P]], channel_multiplier=0)
            iprel = cpool.tile([P, P], dt.int32)
            nc.gpsimd.iota(iprel[:], pattern=[[0, P]], channel_multiplier=1)
            nc.vector.tensor_tensor(
                ident[:], ii[:], iprel[:], op=mybir.AluOpType.is_equal
            )
            identb = cpool.tile([P, P], dt.bfloat16)
            nc.vector.tensor_copy(identb[:], ident[:])
            dgrid = cpool.tile([P, SH], dt.float32)
            for t in range(TILES):
                pt = tppool.tile([P, P], dt.float32, space="PSUM", tag="tp")
                nc.tensor.transpose(
                    out=pt[:],
                    in_=dinv_col[:, t : t + 1].to_broadcast([P, P]),
                    identity=ident[:],
                )
                nc.scalar.activation(dgrid[:, t * P : (t + 1) * P], pt[:], AF.Copy)

            def cload(name, dram, sh):
                t_ = cpool.tile(sh, dt.float32, tag=name)
                nc.sync.dma_start(t_[:], dram[:])
                return t_

            w1f = cload("w1f", W1_d, [F, HID])
            w2f = cload("w2f", W2_d, [HID, HID])
            wxf = cload("wxf", Wx_d, [F, C])
            w1of = cload("w1of", W1o_d, [HID, C])
            w2of = cload("w2of", W2o_d, [HID, C])
            b1c = cload("b1c", b1_d, [HID, 1])
            b2c = cload("b2c", b2_d, [HID, 1])
            boc = cload("boc", bo_d, [C, 1])
            gamc = cload("gamc", gam_d, [HID, 1])
            betc = cload("betc", bet_d, [HID, 1])

            # boundary-folded weights and rank-1 rows
            w2fa = cpool.tile([HID, HID], dt.float32)   # a1*W2
            w1ofa = cpool.tile([HID, C], dt.float32)    # a1*W1o
            w2ofa = cpool.tile([HID, C], dt.float32)    # a2*W2o
            czrow = cpool.tile([1, HID], dt.float32)    # bv1^T W2
            chrow = cpool.tile([1, C], dt.float32)      # bv1^T W1o + bv2^T W2o
            c1orow = cpool.tile([1, C], dt.float32)
            aff = [cpool.tile([HID, 2], dt.float32, tag=f"aff{l}",
                              name=f"aff{l}") for l in range(2)]

            zeroc = cpool.tile([HID, 1], dt.float32)
            nc.gpsimd.memset(zeroc[:], 0.0)
            epscc = cpool.tile([HID, 1], dt.float32)
            nc.gpsimd.memset(epscc[:], BN_EPS)
            statbuf = cpool.tile([HID, TILES], dt.float32, tag="stb1")
            statbuf2 = cpool.tile([HID, TILES], dt.float32, tag="stb2")

            gloc = [None] + [
                dpool.tile([SH, F], dt.bfloat16, name=f"gloc{l}",
                           tag=f"gloc{l}") for l in range(1, 3)
            ]
            # Layer 0's M table (dinv*x) is host-known, so the host ships the
            # FULL replicated table and layer 0 needs no AllGather at all:
            # gathers read m0full_d (IO reads are fine for DMA, just not for
            # collectives) and the self-loop reads the per-core m0_d.
            gsrc = [m0_d, gloc[1], gloc[2]]
            gfull = [None] + [
                dpool.tile([NPAD, F], dt.bfloat16, name=f"gfull{l}",
                           tag=f"gfull{l}", addr_space="Shared")
                for l in range(1, 3)
            ]
            gatherfull = [m0full_d, gfull[1], gfull[2]]
            u_dram = [dpool.tile([P, SH], dt.float32, name=f"u{l}",
                                 tag=f"u{l}") for l in range(2)]

            nidx_regs = {
                w: nc.gpsimd.to_reg(w * P) for w in range(1, MAXCH + 1)
            }

            # ---------------- helpers ----------------
            def emit_M_tile(l, Mf, t):
                """Transpose M tile (bf16 feature-major) -> gloc[l] rows."""
                ptp = tppool.tile([P, P], dt.bfloat16, space="PSUM", tag="tpb")
                nc.tensor.transpose(out=ptp, in_=Mf, identity=identb[:])
                rows = epool.tile([P, P], dt.bfloat16, tag="rows")
                nc.scalar.activation(rows[:], ptp[:], AF.Copy)
                nc.sync.dma_start(gloc[l][t * P : (t + 1) * P, :], rows[:])

            def launch_ag(l):
                nc.gpsimd.collective_compute(
                    "AllGather",
                    mybir.AluOpType.bypass,
                    replica_groups=groups,
                    ins=[gsrc[l][:]],
                    outs=[gfull[l][:]],
                )

            def aggregate_tile(l, t, pieces):
                def piece_for(j, q):
                    key = (q, (j - R0[q]) // MAXCH)
                    if key not in pieces:
                        a = R0[q] + key[1] * MAXCH
                        w = min(MAXCH, REND[q] - a)
                        qn, slot = piece_qs[a]
                        g = gpool.tile(
                            [P, MAXCH * F], dt.bfloat16, tag="g",
                            name=f"g{l}_{q}_{key[1]}",
                        )
                        nc.gpsimd.dma_gather(
                            out_ap=g[:, : w * F].rearrange(
                                "p (k f) -> p k f", k=w
                            ),
                            in_ap=gatherfull[l][
                                q * WIN : (q + 1) * WIN, :
                            ],
                            idxs_ap=idxs[:, SLOTW * slot : SLOTW * slot + 8 * w],
                            num_idxs=w * P,
                            num_idxs_reg=nidx_regs[w],
                            elem_size=F,
                            queue_num=qn,
                        )
                        pieces[key] = (g, a)
                    return pieces[key]

                cols = slice(t * P, (t + 1) * P)
                kt = int(K2[t].sum()) + 1            # +1: self-loop matmul
                acc = apool.tile([F, P], dt.float32, space="PSUM", tag="acc")
                mloc = mpool.tile([P, F], dt.bfloat16, tag="mloc")
                nc.scalar.dma_start(mloc[:], gsrc[l][t * P : (t + 1) * P, :])
                nc.tensor.matmul(
                    out=acc[:, :], lhsT=mloc[:], rhs=identb[:],
                    start=True, stop=(kt == 1),
                )
                mm = 1
                for q in range(Q):
                    kr = int(K2[t, q])
                    if kr == 0:
                        continue
                    j0 = int(J2[t, q])
                    st_ = spool.tile([P, KMAX * P], dt.bfloat16, tag="s")
                    nc.vector.tensor_tensor(
                        st_[:, : kr * P].rearrange("p (g c) -> p g c", g=kr),
                        dsrs[:, j0 : j0 + kr].to_broadcast([P, kr, P]),
                        iota_f[:, : kr * P].rearrange("p (g c) -> p g c", g=kr),
                        op=mybir.AluOpType.is_equal,
                    )
                    for k in range(kr):
                        g, a = piece_for(j0 + k, q)
                        o = j0 + k - a
                        nc.tensor.matmul(
                            out=acc[:, :],
                            lhsT=g[:, o * F : (o + 1) * F],
                            rhs=st_[:, k * P : (k + 1) * P],
                            start=False,
                            stop=(mm == kt - 1),
                        )
                        mm += 1

                # epilogue: uM = acc * dinv_dst (f32)
                uM = epool.tile([P, P], dt.float32, tag="uM")
                nc.vector.tensor_tensor(
                    uM[:], acc[:, :], dgrid[:, cols], op=mybir.AluOpType.mult
                )
                if l < 2:
                    nc.sync.dma_start(u_dram[l][:, cols], uM[:])
                    zp = zppool.tile([HID, P], dt.float32, space="PSUM",
                                     tag="zp")
                    if l == 0:
                        nc.tensor.matmul(
                            out=zp[:], lhsT=w1f[:], rhs=uM[:],
                            start=True, stop=True,
                        )
                        zbias = b1c
                    else:
                        nc.tensor.matmul(
                            out=zp[:], lhsT=w2fa[:], rhs=uM[:],
                            start=True, stop=False,
                        )
                        wrt = epool.tile([1, P], dt.float32, tag="wrt")
                        nc.sync.dma_start(wrt[:], wrow_d[0:1, cols])
                        nc.tensor.matmul(
                            out=zp[:], lhsT=czrow[:], rhs=wrt[:],
                            start=False, stop=True,
                        )
                        zbias = b2c
                    zt = epool.tile([HID, P], dt.float32, tag="zt")
                    nc.scalar.activation(
                        zt[:], zp[:], AF.Relu, bias=zbias[:, 0:1],
                        accum_out=statbuf[:, t : t + 1],
                    )
                    sq = epool.tile([HID, P], dt.float32, tag="sq")
                    nc.scalar.activation(
                        sq[:], zt[:], AF.Square,
                        accum_out=statbuf2[:, t : t + 1],
                    )
                    Mf = epool.tile([P, P], dt.bfloat16, tag="Mf")
                    nc.vector.tensor_tensor(
                        Mf[:], zt[:], dgrid[:, cols], op=mybir.AluOpType.mult
                    )
                    emit_M_tile(l + 1, Mf[:], t)
                else:
                    u0t = epool.tile([P, P], dt.float32, tag="u0t")
                    nc.scalar.dma_start(u0t[:], u_dram[0][:, cols])
                    u1t = epool.tile([P, P], dt.float32, tag="u1t")
                    nc.scalar.dma_start(u1t[:], u_dram[1][:, cols])
                    wrt = epool.tile([1, P], dt.float32, tag="wrt")
                    nc.sync.dma_start(wrt[:], wrow_d[0:1, cols])
                    zp = zppool.tile([C, P], dt.float32, space="PSUM", tag="zp")
                    nc.tensor.matmul(
                        out=zp[:], lhsT=wxf[:], rhs=u0t[:],
                        start=True, stop=False,
                    )
                    nc.tensor.matmul(
                        out=zp[:], lhsT=w1ofa[:], rhs=u1t[:],
                        start=False, stop=False,
                    )
                    nc.tensor.matmul(
                        out=zp[:], lhsT=w2ofa[:], rhs=uM[:],
                        start=False, stop=False,
                    )
                    nc.tensor.matmul(
                        out=zp[:], lhsT=chrow[:], rhs=wrt[:],
                        start=False, stop=True,
                    )
                    z3 = epool.tile([C, P], dt.float32, tag="z3")
                    nc.scalar.activation(
                        z3[:], zp[:], AF.Relu, bias=boc[:, 0:1]
                    )
                    ptp = tppool.tile([P, C], dt.float32, space="PSUM", tag="tp")
                    nc.tensor.transpose(
                        out=ptp[:], in_=z3[:], identity=ident[:C, :C]
                    )
                    onm = epool.tile([P, C], dt.float32, tag="onm")
                    nc.scalar.activation(onm[:], ptp[:], AF.Copy)
                    nc.sync.dma_start(out_d[t * P : (t + 1) * P, :], onm[:])

            def boundary_ar(l):
                """Launch the 1KB stats AllReduce (before the AllGather so it
                clears the in-order CC queue first and the folds overlap the
                AllGather)."""
                stl = dpool.tile([HID, 2], dt.float32, tag=f"stl{l}")
                sts = wpool.tile([HID, 2], dt.float32, tag="w2")
                nc.vector.reduce_sum(
                    sts[:, 0:1], statbuf[:], axis=mybir.AxisListType.X
                )
                nc.vector.reduce_sum(
                    sts[:, 1:2], statbuf2[:], axis=mybir.AxisListType.X
                )
                nc.sync.dma_start(stl[:], sts[:])
                star = dpool.tile([HID, 2], dt.float32, tag=f"star{l}")
                nc.gpsimd.collective_compute(
                    "AllReduce",
                    mybir.AluOpType.add,
                    replica_groups=groups,
                    ins=[stl[:]],
                    outs=[star[:]],
                )
                return star

            def boundary(l, star):
                """Stats -> a, bv; fold a into next weights; bv -> c-rows.
                Runs on gpsimd/scalar only: any op here waits on the
                AllReduce, and an AR-gated op in the DVE stream would
                head-of-line block the next layer's selection builds for the
                whole collective barrier."""
                zbias = b1c if l == 0 else b2c
                af = aff[l]
                stg = wpool.tile([HID, 2], dt.float32, tag="w2")
                nc.sync.dma_start(stg[:], star[:])
                rb = wpool.tile([HID, 2], dt.float32, tag="w0")
                nc.scalar.activation(
                    rb[:, 0:1], zeroc[:], AF.Relu, bias=zbias[:, 0:1]
                )
                nc.scalar.activation(rb[:, 1:2], rb[:, 0:1], AF.Square)
                corr = wpool.tile([HID, 2], dt.float32, tag="w1")
                nc.gpsimd.tensor_scalar(
                    out=corr[:], in0=rb[:], scalar1=-float(NPAD - N),
                    scalar2=None, op0=mybir.AluOpType.mult,
                )
                nc.gpsimd.tensor_add(stg[:], stg[:], corr[:])
                mv = wpool.tile([HID, 2], dt.float32, tag="w3")
                nc.gpsimd.tensor_scalar(
                    out=mv[:], in0=stg[:], scalar1=1.0 / N, scalar2=None,
                    op0=mybir.AluOpType.mult,
                )
                m2 = wpool.tile([HID, 1], dt.float32, tag="w0")
                nc.gpsimd.tensor_tensor(
                    m2[:], mv[:, 0:1], mv[:, 0:1], op=mybir.AluOpType.mult
                )
                var = wpool.tile([HID, 1], dt.float32, tag="w1")
                nc.gpsimd.tensor_sub(var[:], mv[:, 1:2], m2[:])
                # 1/sqrt(var+eps) = exp(-0.5*ln(var+eps)): keeps the whole
                # fold chain off the DVE stream (vector.reciprocal here would
                # head-of-line block the next layer's selection builds).
                lnv = wpool.tile([HID, 1], dt.float32, tag="w3")
                nc.scalar.activation(
                    lnv[:], var[:], AF.Ln, bias=epscc[:, 0:1]
                )
                nc.gpsimd.tensor_scalar(
                    out=lnv[:], in0=lnv[:], scalar1=-0.5, scalar2=None,
                    op0=mybir.AluOpType.mult,
                )
                sd = wpool.tile([HID, 1], dt.float32, tag="w1")
                nc.scalar.activation(sd[:], lnv[:], AF.Exp)
                nc.gpsimd.tensor_tensor(
                    af[:, 0:1], gamc[:], sd[:], op=mybir.AluOpType.mult
                )
                am = wpool.tile([HID, 1], dt.float32, tag="w0")
                nc.gpsimd.tensor_tensor(
                    am[:], af[:, 0:1], mv[:, 0:1], op=mybir.AluOpType.mult
                )
                nc.gpsimd.tensor_sub(af[:, 1:2], betc[:], am[:])

                if l == 0:
                    nc.gpsimd.tensor_scalar(
                        out=w2fa[:], in0=w2f[:], scalar1=af[:, 0:1],
                        scalar2=None, op0=mybir.AluOpType.mult,
                    )
                    nc.gpsimd.tensor_scalar(
                        out=w1ofa[:], in0=w1of[:], scalar1=af[:, 0:1],
                        scalar2=None, op0=mybir.AluOpType.mult,
                    )
                    czp = zppool.tile([1, HID], dt.float32, space="PSUM",
                                      tag="zp")
                    nc.tensor.matmul(
                        out=czp[:], lhsT=af[:, 1:2], rhs=w2f[:],
                        start=True, stop=True,
                    )
                    nc.scalar.activation(czrow[:], czp[:], AF.Copy)
                    c1p = zppool.tile([1, C], dt.float32, space="PSUM",
                                      tag="zp")
                    nc.tensor.matmul(
                        out=c1p[:], lhsT=af[:, 1:2], rhs=w1of[:],
                        start=True, stop=True,
                    )
                    nc.scalar.activation(c1orow[:], c1p[:], AF.Copy)
                else:
                    nc.gpsimd.tensor_scalar(
                        out=w2ofa[:], in0=w2of[:], scalar1=af[:, 0:1],
                        scalar2=None, op0=mybir.AluOpType.mult,
                    )
                    c2p = zppool.tile([1, C], dt.float32, space="PSUM",
                                      tag="zp")
                    nc.tensor.matmul(
                        out=c2p[:], lhsT=af[:, 1:2], rhs=w2of[:],
                        start=True, stop=True,
                    )
                    c2s = wpool.tile([1, C], dt.float32, tag="w0")
                    nc.scalar.activation(c2s[:], c2p[:], AF.Copy)
                    nc.gpsimd.tensor_add(chrow[:], c1orow[:], c2s[:])

            # ---------------- main schedule ----------------
            for l in range(3):
                pieces = {}
                for t in range(TILES):
                    aggregate_tile(l, t, pieces)
                if l < 2:
                    star = boundary_ar(l)
                    launch_ag(l + 1)
                    boundary(l, star)

    from concourse import mybir as _mybir

    nc.compile()
    if not skip_wait_split:
        _split_excess_waits(nc, _mybir, bass_rust, max_waits=1)
    return nc


def make_in_maps(x, edge_index, W1, b1, W2, b2, Wout, bout, gamma, beta):
    import ml_dtypes

    x = np.asarray(x, dtype=np.float32)
    edge_index = np.asarray(edge_index)
    idx16, dstrel, K2, deg, wvec, slot = _prep_edges(edge_index)

    dinv = 1.0 / np.sqrt(deg)
    m0p = np.zeros((S, SH, F), dtype=ml_dtypes.bfloat16)
    deg_cols = np.zeros((S, P, TILES), dtype=np.float32)
    wrows = np.zeros((S, 1, SH), dtype=np.float32)
    for c in range(S):
        sl = slot[c * REAL : (c + 1) * REAL]
        degp_c = np.full(SH, 1.0e30, dtype=np.float32)
        degp_c[sl] = deg[c * REAL : (c + 1) * REAL]
        deg_cols[c] = degp_c.reshape(TILES, P).T
        wrows[c, 0, sl] = wvec[c * REAL : (c + 1) * REAL]
        m0p[c][sl] = (
            x[c * REAL : (c + 1) * REAL]
            * dinv[c * REAL : (c + 1) * REAL, None]
        ).astype(ml_dtypes.bfloat16)
    m0full = np.ascontiguousarray(m0p.reshape(NPAD, F))

    W1 = np.asarray(W1, np.float32)
    W2 = np.asarray(W2, np.float32)
    Wout = np.asarray(Wout, np.float32)

    in_maps = []
    for c in range(S):
        in_maps.append(
            {
                "m0": np.ascontiguousarray(m0p[c]),
                "m0full": m0full,
                "idx16": idx16[c],
                "dstrel": dstrel[c],
                "deg": np.ascontiguousarray(deg_cols[c]),
                "wrow": np.ascontiguousarray(wrows[c]),
                "W1": W1,
                "W2": W2,
                "Wx": np.ascontiguousarray(Wout[0:F]),
                "W1o": np.ascontiguousarray(Wout[F : F + HID]),
                "W2o": np.ascontiguousarray(Wout[F + HID :]),
                "b1": np.asarray(b1, np.float32).reshape(-1, 1),
                "b2": np.asarray(b2, np.float32).reshape(-1, 1),
                "bout": np.asarray(bout, np.float32).reshape(-1, 1),
                "gamma": np.asarray(gamma, np.float32).reshape(-1, 1),
                "beta": np.asarray(beta, np.float32).reshape(-1, 1),
            }
        )
    return in_maps, K2, slot


_CACHE = {}
LAST_RESULT = None


def kernel(x, edge_index, W1, b1, W2, b2, Wout, bout, gamma, beta):
    global LAST_RESULT
    import os
    from concourse.bass_utils import run_bass_kernel_spmd

    in_maps, K2, slot = make_in_maps(
        x, edge_index, W1, b1, W2, b2, Wout, bout, gamma, beta
    )
    key = tuple(K2.ravel().tolist())
    if key not in _CACHE:
        _CACHE[key] = _build_program(K2)
    nc = _CACHE[key]

    trace = os.environ.get("GNN_TRACE", "") == "1"
    tmpdir = os.environ.get("GNN_TMPDIR") or None
    if tmpdir:
        os.makedirs(tmpdir, exist_ok=True)
    res = run_bass_kernel_spmd(
        nc, in_maps, list(range(S)), trace=trace, tmpdir=tmpdir
    )
    LAST_RESULT = res
    out = np.empty((N, C), dtype=np.float32)
    for c in range(S):
        sl = slot[c * REAL : (c + 1) * REAL]
        out[c * REAL : (c + 1) * REAL] = res.results[c]["out"][sl]
    return out

